# revision 1
# baseline (speedup 1.0000x reference)
"""Cross-attention kernel for Trainium2, sharded over 8 NeuronCores.

Shards query rows across cores (1024 rows each); K/V work is replicated.
All matmuls run with fp16 operands (1 cycle/row on the PE, 4x faster than
fp32) accumulating in fp32 PSUM.  Raw q/k/v/W are cast fp32->fp16 during the
SWDGE load DMA and transposed on-chip with the DMA xbar (2-byte dtypes only),
so no compute engine spends cycles on layout.

Algebraic simplifications:
  - bk is dropped: Q @ (bk x 1)^T adds a row-constant to the scores, which
    softmax cancels.
  - softmax normalization and the post-softmax 1/sqrt(dk) scale are folded
    into one per-row multiply of the final PV product.
  - scores are stored as (chunkmax - s) in fp16; the error of the fp16
    representation vanishes for the entries near the max, which are the only
    ones softmax keeps.
"""
import sys

sys.path.insert(0, "/opt/trn_rl_repo")

import numpy as np  # noqa: E402
import concourse.bass as bass  # noqa: E402
import concourse.tile as tile  # noqa: E402
from concourse import mybir  # noqa: E402
from concourse import bass_utils  # noqa: E402
from contextlib import ExitStack  # noqa: E402

F16 = mybir.dt.float16
F32 = mybir.dt.float32
AF = mybir.ActivationFunctionType
AX = mybir.AxisListType
ALU = mybir.AluOpType

P = 128
D = 1024            # input dim
ND = D // P         # 8 d-chunks
C = 512             # dim_k
NCC = C // P        # 4 c-chunks
VD = 512            # dim_v
KEYS = 8192
NSLAB = KEYS // 512  # 16 slabs of 512 keys
RL = 1024           # rows per core
NB = RL // P        # 8 row blocks
NCORES = 8
NORM = float(1.0 / np.sqrt(np.float32(C)))

_ws_counter = [0]


def _split_multi_waits(nc):
    """This container's walrus accepts only ONE sync-wait per instruction.
    Move extra waits onto preceding same-engine EventSemaphore insts."""
    for f in nc.m.functions:
        for bb in f.blocks:
            il = bb.instructions
            if not any(
                inst.sync_info is not None and len(inst.sync_info.on_wait or ()) > 1
                for inst in il
            ):
                continue
            new = []
            for inst in il:
                si = inst.sync_info
                if si is not None and len(si.on_wait or ()) > 1:
                    waits = list(si.on_wait)
                    for w in waits[:-1]:
                        _ws_counter[0] += 1
                        new.append(
                            mybir.InstEventSemaphore(
                                name=f"I-ws{_ws_counter[0]}",
                                engine=inst.engine,
                                ins=[],
                                outs=[],
                                sync_info=mybir.SyncInfo(on_wait=[w], on_update=[]),
                            )
                        )
                    del si.on_wait[:-1]
                new.append(inst)
            bb.instructions = new


def _emit(nc, tc, aps):
    q_ap, k_ap, v_ap = aps["q"], aps["k"], aps["v"]
    wq_ap, wk_ap, wv_ap = aps["wq"], aps["wk"], aps["wv"]
    bq_ap, bv_ap = aps["bq"], aps["bv"]
    out_ap = aps["out"]

    with ExitStack() as top:
        const = top.enter_context(tc.tile_pool(name="const", bufs=1))
        big = top.enter_context(tc.tile_pool(name="big", bufs=1))

        ones = const.tile([1, P], F16, tag="ones")
        nc.vector.memset(ones[:], 1.0)
        bv16 = const.tile([1, VD], F16, tag="bv16")
        nc.gpsimd.dma_start(bv16[:], bv_ap[None, :])
        bqT = const.tile([P, NCC], F32, tag="bqT")
        nc.scalar.dma_start(bqT[:], bq_ap.rearrange("(j p) -> p j", p=P))

        # Long-lived operand stores (fp16):
        KT_sb = [big.tile([P, KEYS], F16, tag=f"KT{ci}", name=f"KT{ci}") for ci in range(NCC)]
        V_sb = [big.tile([P, 16 * VD], F16, tag=f"V{g}", name=f"V{g}") for g in range(4)]
        QT_sb = [big.tile([P, RL], F16, tag=f"QT{ci}", name=f"QT{ci}") for ci in range(NCC)]

        with ExitStack() as proj:
            wts = proj.enter_context(tc.tile_pool(name="wts", bufs=1))
            dram = proj.enter_context(tc.tile_pool(name="dram", bufs=1, space="DRAM"))
            xt = proj.enter_context(tc.tile_pool(name="xt", bufs=20))
            psp = proj.enter_context(tc.tile_pool(name="psp", bufs=4, space="PSUM"))

            # ---- stage fp16 copies in DRAM (SWDGE cast), then transpose with
            # ---- few, large xbar ops ([rows,128] DRAM -> [128,rows] SBUF) ----
            WT = {}
            for wname, w_ap in (("wq", wq_ap), ("wk", wk_ap), ("wv", wv_ap)):
                w16 = dram.tile([C, D], F16, tag=f"{wname}16", name=f"{wname}16")
                nc.gpsimd.dma_start(w16[:], w_ap[:])
                wt_d = [wts.tile([P, C], F16, tag=f"{wname}T{d}", name=f"{wname}T{d}") for d in range(ND)]
                for d in range(ND):
                    nc.sync.dma_start(wt_d[d][:], w16[:, d * P:(d + 1) * P],
                                      transpose=True)
                WT[wname] = wt_d

            # ---- Q projection: QT_sb[ci][:, rows] = Wq @ q^T + bq ----
            q16 = dram.tile([RL, D], F16, tag="q16", name="q16")
            nc.gpsimd.dma_start(q16[:], q_ap[:])
            qT = []
            for d in range(ND):
                t = xt.tile([P, RL], F16, tag="xt", name="qT_t")
                nc.sync.dma_start(t[:], q16[:, d * P:(d + 1) * P], transpose=True)
                qT.append(t)
            for ci in range(NCC):
                for rh in range(2):
                    ps = psp.tile([P, 512], F32, tag="psp")
                    for d in range(ND):
                        nc.tensor.matmul(
                            ps[:],
                            WT["wq"][d][:, ci * P:(ci + 1) * P],
                            qT[d][:, rh * 512:(rh + 1) * 512],
                            start=(d == 0),
                            stop=(d == ND - 1),
                        )
                    nc.scalar.activation(
                        QT_sb[ci][:, rh * 512:(rh + 1) * 512],
                        ps[:],
                        AF.Identity,
                        bias=bqT[:, ci:ci + 1],
                        scale=1.0,
                    )
            qT = None

            # ---- K/V projections, streamed by 1024-key group ----
            NG = KEYS // RL  # 8 groups of 1024 keys
            k16 = []
            v16 = []
            for g in range(NG):
                t = dram.tile([RL, D], F16, tag=f"k16_{g}", name=f"k16_{g}")
                nc.gpsimd.dma_start(t[:], k_ap[g * RL:(g + 1) * RL, :])
                k16.append(t)
                t = dram.tile([RL, D], F16, tag=f"v16_{g}", name=f"v16_{g}")
                nc.gpsimd.dma_start(t[:], v_ap[g * RL:(g + 1) * RL, :])
                v16.append(t)
            for g in range(NG):
                kT, vT = [], []
                for d in range(ND):
                    t = xt.tile([P, RL], F16, tag="xt", name="kT_t")
                    nc.sync.dma_start(t[:], k16[g][:, d * P:(d + 1) * P],
                                      transpose=True)
                    kT.append(t)
                    t = xt.tile([P, RL], F16, tag="xt", name="vT_t")
                    nc.sync.dma_start(t[:], v16[g][:, d * P:(d + 1) * P],
                                      transpose=True)
                    vT.append(t)
                # K^T[c, keys] (no bias: bk cancels in softmax)
                for sc in range(2):
                    s = g * 2 + sc
                    for ci in range(NCC):
                        ps = psp.tile([P, 512], F32, tag="psp")
                        for d in range(ND):
                            nc.tensor.matmul(
                                ps[:],
                                WT["wk"][d][:, ci * P:(ci + 1) * P],
                                kT[d][:, sc * 512:(sc + 1) * 512],
                                start=(d == 0),
                                stop=(d == ND - 1),
                            )
                        nc.scalar.copy(KT_sb[ci][:, s * 512:(s + 1) * 512], ps[:])
                # V[keys, v] with bv via rank-1 ones matmul
                for j in range(8):
                    ps = psp.tile([P, 512], F32, tag="psp")
                    nc.tensor.matmul(ps[:], ones[:], bv16[:], start=True, stop=False)
                    for d in range(ND):
                        nc.tensor.matmul(
                            ps[:],
                            vT[d][:, j * P:(j + 1) * P],
                            WT["wv"][d][:],
                            start=False,
                            stop=(d == ND - 1),
                        )
                    kc = g * 8 + j
                    nc.scalar.copy(
                        V_sb[kc // 16][:, (kc % 16) * VD:(kc % 16 + 1) * VD], ps[:]
                    )

        # ---- attention, one 128-row block at a time ----
        with ExitStack() as att:
            spool = att.enter_context(tc.tile_pool(name="spool", bufs=2))
            apool = att.enter_context(tc.tile_pool(name="apool", bufs=6))
            atp = att.enter_context(tc.tile_pool(name="atp", bufs=16))
            stat = att.enter_context(tc.tile_pool(name="stat", bufs=2))
            outp = att.enter_context(tc.tile_pool(name="outp", bufs=3))
            pss = att.enter_context(tc.tile_pool(name="pss", bufs=3, space="PSUM"))
            pso = att.enter_context(tc.tile_pool(name="pso", bufs=2, space="PSUM"))

            for b in range(NB):
                S = spool.tile([P, KEYS], F16, tag="S")  # stores chunkmax - s
                cm = stat.tile([P, NSLAB], F32, tag="cm")
                for sc in range(NSLAB):
                    ps = pss.tile([P, 512], F32, tag="pss")
                    for ci in range(NCC):
                        nc.tensor.matmul(
                            ps[:],
                            QT_sb[ci][:, b * P:(b + 1) * P],
                            KT_sb[ci][:, sc * 512:(sc + 1) * 512],
                            start=(ci == 0),
                            stop=(ci == NCC - 1),
                        )
                    nc.vector.reduce_max(cm[:, sc:sc + 1], ps[:], axis=AX.X)
                    # S' = chunkmax - s  (>= 0; near-max entries keep full precision)
                    nc.scalar.activation(
                        S[:, sc * 512:(sc + 1) * 512],
                        ps[:],
                        AF.Identity,
                        bias=cm[:, sc:sc + 1],
                        scale=-1.0,
                    )
                rm = stat.tile([P, 1], F32, tag="rm")
                nc.vector.reduce_max(rm[:], cm[:], axis=AX.X)
                bias_mat = stat.tile([P, NSLAB], F32, tag="bias")
                nc.vector.tensor_scalar(
                    bias_mat[:], cm[:], rm[:], None, op0=ALU.subtract
                )
                csum = stat.tile([P, NSLAB], F32, tag="csum")
                AT = []
                for sc in range(NSLAB):
                    A = apool.tile([P, 512], F16, tag="A")
                    # exp(-(S') + (cm - rm)) = exp(s - rowmax)
                    nc.scalar.activation(
                        A[:],
                        S[:, sc * 512:(sc + 1) * 512],
                        AF.Exp,
                        bias=bias_mat[:, sc:sc + 1],
                        scale=-1.0,
                        accum_out=csum[:, sc:sc + 1],
                    )
                    for jj in range(4):
                        t = atp.tile([P, P], F16, tag="AT", name="AT_t")
                        nc.sync.dma_start(
                            t[:], A[:, jj * P:(jj + 1) * P], transpose=True
                        )
                        AT.append(t)
                rs = stat.tile([P, 1], F32, tag="rs")
                nc.vector.reduce_sum(rs[:], csum[:], axis=AX.X)
                rinv = stat.tile([P, 1], F32, tag="rinv")
                nc.vector.reciprocal(rinv[:], rs[:])

                po = pso.tile([P, VD], F32, tag="pso")
                for kc in range(64):
                    nc.tensor.matmul(
                        po[:],
                        AT[kc][:],
                        V_sb[kc // 16][:, (kc % 16) * VD:(kc % 16 + 1) * VD],
                        start=(kc == 0),
                        stop=(kc == 63),
                    )
                out_sb = outp.tile([P, VD], F32, tag="out")
                nc.vector.tensor_scalar(
                    out_sb[:], po[:], rinv[:], NORM, op0=ALU.mult, op1=ALU.mult
                )
                nc.scalar.dma_start(out_ap[b * P:(b + 1) * P, :], out_sb[:])


_cached = {}


def _build():
    if "nc" in _cached:
        return _cached["nc"]
    nc = bass.Bass("TRN2", target_bir_lowering=False, debug=False)
    aps = {
        "q": nc.dram_tensor("q", [RL, D], F32, kind="ExternalInput").ap(),
        "k": nc.dram_tensor("k", [KEYS, D], F32, kind="ExternalInput").ap(),
        "v": nc.dram_tensor("v", [KEYS, D], F32, kind="ExternalInput").ap(),
        "wq": nc.dram_tensor("wq", [C, D], F32, kind="ExternalInput").ap(),
        "wk": nc.dram_tensor("wk", [C, D], F32, kind="ExternalInput").ap(),
        "wv": nc.dram_tensor("wv", [C, D], F32, kind="ExternalInput").ap(),
        "bq": nc.dram_tensor("bq", [C], F32, kind="ExternalInput").ap(),
        "bv": nc.dram_tensor("bv", [VD], F32, kind="ExternalInput").ap(),
        "out": nc.dram_tensor("out", [RL, VD], F32, kind="ExternalOutput").ap(),
    }
    with tile.TileContext(nc) as tc:
        _emit(nc, tc, aps)
    _split_multi_waits(nc)
    _cached["nc"] = nc
    return nc


def kernel(q, k, v, Wq, bq, Wk, bk, Wv, bv, _trace=False, _tmpdir=None):
    del bk  # provably cancels inside the softmax
    nc = _build()
    f32 = lambda a: np.ascontiguousarray(np.asarray(a, dtype=np.float32))
    q, k, v = f32(q), f32(k), f32(v)
    base = {
        "k": k, "v": v, "wq": f32(Wq), "wk": f32(Wk), "wv": f32(Wv),
        "bq": f32(bq), "bv": f32(bv),
    }
    in_maps = [
        dict(base, q=np.ascontiguousarray(q[c * RL:(c + 1) * RL]))
        for c in range(NCORES)
    ]
    res = bass_utils.run_bass_kernel_spmd(
        nc, in_maps, core_ids=list(range(NCORES)), trace=_trace, tmpdir=_tmpdir
    )
    out = np.concatenate([res.results[c]["out"] for c in range(NCORES)], axis=0)
    if _trace:
        kernel.last_results = res
    return out



# revision 2
# speedup vs baseline: 1.3005x; 1.3005x over previous
"""Cross-attention kernel for Trainium2, sharded over 8 NeuronCores.

Shards query rows across cores (1024 rows each); K/V work is replicated.

Structure (flash-attention streaming, one pass over 8 key-groups of 1024):
  - Host pre-marshals inputs: fp32->fp16 cast + transpose into the exact
    [contraction-on-partition] layouts the PE needs.  The device does zero
    layout work for inputs; all FLOPs (projections + attention) stay on
    device.  Device DMA-in is ~41MB/core instead of the ~150MB a DRAM
    staging round-trip costs.
  - Per group g: load kT/vT slab (gpsimd SWDGE, double-buffered), project
    K^T [c,keys] and V [keys,v] (PE, evicted fp16 by the scalar engine),
    then stream attention for all 8 query row-blocks: S = Q@K^T (PSUM),
    online-softmax rescale (DVE/Act), A^T via DMA-xbar transpose, and
    PV accumulated into per-block fp32 accumulators (flash rescale by
    exp(m_old - m_new) via one fused scalar_tensor_tensor).
  - QK and PV are software-pipelined (depth 3) so the PE never waits for
    the softmax/transpose of the block it just produced.
  - PSUM budget exactly 8 banks: 2 proj + 2x2 scores + 2 PV.

Algebraic simplifications:
  - bk is dropped: it adds a per-row constant to scores, softmax cancels it.
  - bv is folded into the epilogue: atten rows sum to NORM after scaling,
    so out += NORM * bv.
  - softmax normalization and the post-softmax 1/sqrt(dk) scale fold into
    one per-row multiply at the end.
"""
import sys

sys.path.insert(0, "/opt/trn_rl_repo")

import numpy as np  # noqa: E402
import concourse.bass as bass  # noqa: E402
import concourse.tile as tile  # noqa: E402
from concourse import mybir  # noqa: E402
from concourse import bass_utils  # noqa: E402
from contextlib import ExitStack  # noqa: E402

F16 = mybir.dt.float16
F32 = mybir.dt.float32
AF = mybir.ActivationFunctionType
AX = mybir.AxisListType
ALU = mybir.AluOpType

P = 128
D = 1024             # input dim
ND = D // P          # 8 d-chunks
C = 512              # dim_k
NCC = C // P         # 4 c-chunks
VD = 512             # dim_v
KEYS = 8192
GK = 1024            # keys per group
NG = KEYS // GK      # 8 groups
RL = 1024            # query rows per core
NB = RL // P         # 8 row blocks
NCORES = 8
DEPTH = 3            # QK->PV software pipeline depth
NORM = float(1.0 / np.sqrt(np.float32(C)))

_ws_counter = [0]


def _split_multi_waits(nc):
    """This container's walrus accepts only ONE sync-wait per instruction.
    Move extra waits onto preceding same-engine EventSemaphore insts."""
    for f in nc.m.functions:
        for bb in f.blocks:
            il = bb.instructions
            if not any(
                inst.sync_info is not None and len(inst.sync_info.on_wait or ()) > 1
                for inst in il
            ):
                continue
            new = []
            for inst in il:
                si = inst.sync_info
                if si is not None and len(si.on_wait or ()) > 1:
                    waits = list(si.on_wait)
                    for w in waits[:-1]:
                        _ws_counter[0] += 1
                        new.append(
                            mybir.InstEventSemaphore(
                                name=f"I-ws{_ws_counter[0]}",
                                engine=inst.engine,
                                ins=[],
                                outs=[],
                                sync_info=mybir.SyncInfo(on_wait=[w], on_update=[]),
                            )
                        )
                    del si.on_wait[:-1]
                new.append(inst)
            bb.instructions = new


def _emit(nc, tc, aps):
    qT_r = aps["qT"].rearrange("(dc p) r -> p dc r", p=P)
    kT_r = aps["kT"].rearrange("(dc p) k -> p dc k", p=P)
    vT_r = aps["vT"].rearrange("(dc p) k -> p dc k", p=P)
    out_ap = aps["out"]

    with ExitStack() as top:
        const = top.enter_context(tc.tile_pool(name="const", bufs=1))
        kvin = top.enter_context(tc.tile_pool(name="kvin", bufs=2))
        proj = top.enter_context(tc.tile_pool(name="proj", bufs=2))
        apool = top.enter_context(tc.tile_pool(name="apool", bufs=4))
        atp = top.enter_context(tc.tile_pool(name="atp", bufs=6))
        stat = top.enter_context(tc.tile_pool(name="stat", bufs=6))
        outp = top.enter_context(tc.tile_pool(name="outp", bufs=2))
        pp = top.enter_context(tc.tile_pool(name="pp", bufs=2, space="PSUM"))
        psS = top.enter_context(tc.tile_pool(name="psS", bufs=2, space="PSUM"))
        pop = top.enter_context(tc.tile_pool(name="pop", bufs=2, space="PSUM"))

        # ---- persistent operands ----
        wq = const.tile([P, ND, C], F16, tag="wq")
        wk = const.tile([P, ND, C], F16, tag="wk")
        wv = const.tile([P, ND, C], F16, tag="wv")
        nc.sync.dma_start(wq[:], aps["wqT"].rearrange("(dc p) c -> p dc c", p=P))
        nc.sync.dma_start(wk[:], aps["wkT"].rearrange("(dc p) c -> p dc c", p=P))
        nc.sync.dma_start(wv[:], aps["wvT"].rearrange("(dc p) c -> p dc c", p=P))
        bqT = const.tile([P, NCC], F32, tag="bqT")
        nc.sync.dma_start(bqT[:], aps["bq"].rearrange("(j p) -> p j", p=P))
        bvrow = const.tile([1, VD], F32, tag="bvrow")
        nc.scalar.dma_start(bvrow[:], aps["bv"][None, :])
        ones1 = const.tile([1, P], F32, tag="ones1")
        nc.vector.memset(ones1[:], 1.0)

        # bvN[p, v] = NORM * bv[v] broadcast along partitions (rank-1 matmul)
        bvN = const.tile([P, VD], F32, tag="bvN")
        psb0 = pp.tile([P, VD], F32, tag="pp")
        nc.tensor.matmul(psb0[:], ones1[:], bvrow[:], start=True, stop=True)
        nc.scalar.activation(bvN[:], psb0[:], AF.Copy, bias=0.0, scale=NORM)

        # Q^T projection: QT[c-chunk][128, rows] fp16, bias bq folded in.
        QT = const.tile([P, NCC, RL], F16, tag="QT")
        qin = kvin.tile([P, ND, RL], F16, tag="qin", bufs=1)
        nc.sync.dma_start(qin[:], qT_r[:])
        for ci in range(NCC):
            for rh in range(2):
                ps = pp.tile([P, 512], F32, tag="pp")
                for d in range(ND):
                    nc.tensor.matmul(
                        ps[:],
                        wq[:, d, ci * P:(ci + 1) * P],
                        qin[:, d, rh * 512:(rh + 1) * 512],
                        start=(d == 0),
                        stop=(d == ND - 1),
                    )
                nc.scalar.activation(
                    QT[:, ci, rh * 512:(rh + 1) * 512],
                    ps[:],
                    AF.Identity,
                    bias=bqT[:, ci:ci + 1],
                    scale=1.0,
                )

        # flash state, ping-pong by group parity: IN = st[g%2], OUT = st[1-g%2]
        m_st = [const.tile([P, NB], F32, tag=f"m{i}", name=f"m{i}") for i in range(2)]
        rs_st = [const.tile([P, NB], F32, tag=f"rs{i}", name=f"rs{i}") for i in range(2)]
        O_st = [
            const.tile([P, NB, VD], F32, tag=f"O{i}", name=f"O{i}") for i in range(2)
        ]

        def load_group(g):
            kt = kvin.tile([P, ND, GK], F16, tag="kT", name=f"kTg{g}")
            vt = kvin.tile([P, ND, GK], F16, tag="vT", name=f"vTg{g}")
            nc.gpsimd.dma_start(kt[:], kT_r[:, :, g * GK:(g + 1) * GK])
            nc.gpsimd.dma_start(vt[:], vT_r[:, :, g * GK:(g + 1) * GK])
            return kt, vt

        nxt = load_group(0)
        for g in range(NG):
            kt, vt = nxt
            if g + 1 < NG:
                nxt = load_group(g + 1)
            m_in, m_out = m_st[g % 2], m_st[1 - g % 2]
            rs_in, rs_out = rs_st[g % 2], rs_st[1 - g % 2]
            O_in, O_out = O_st[g % 2], O_st[1 - g % 2]

            # ---- K^T / V projections for this group ----
            KT = proj.tile([P, NCC, GK], F16, tag="KT", name=f"KTg{g}")
            Vg = proj.tile([P, ND, VD], F16, tag="V", name=f"Vg{g}")
            for ci in range(NCC):
                for h in range(2):
                    ps = pp.tile([P, 512], F32, tag="pp")
                    for d in range(ND):
                        nc.tensor.matmul(
                            ps[:],
                            wk[:, d, ci * P:(ci + 1) * P],
                            kt[:, d, h * 512:(h + 1) * 512],
                            start=(d == 0),
                            stop=(d == ND - 1),
                        )
                    nc.scalar.copy(KT[:, ci, h * 512:(h + 1) * 512], ps[:])
            for kc in range(8):
                ps = pp.tile([P, 512], F32, tag="pp")
                for d in range(ND):
                    nc.tensor.matmul(
                        ps[:],
                        vt[:, d, kc * P:(kc + 1) * P],
                        wv[:, d, :],
                        start=(d == 0),
                        stop=(d == ND - 1),
                    )
                nc.scalar.copy(Vg[:, kc, :], ps[:])

            # ---- attention over this group, pipelined across row blocks ----
            at_h = {}
            f_h = {}

            def qk(b):
                S0 = psS.tile([P, 512], F32, tag="S0", name=f"S0_{g}_{b}")
                S1 = psS.tile([P, 512], F32, tag="S1", name=f"S1_{g}_{b}")
                for S, h in ((S0, 0), (S1, 1)):
                    for ci in range(NCC):
                        nc.tensor.matmul(
                            S[:],
                            QT[:, ci, b * P:(b + 1) * P],
                            KT[:, ci, h * 512:(h + 1) * 512],
                            start=(ci == 0),
                            stop=(ci == NCC - 1),
                        )
                gm0 = stat.tile([P, 1], F32, tag="gm0", name="gm0")
                gm1 = stat.tile([P, 1], F32, tag="gm1", name="gm1")
                nc.vector.reduce_max(gm0[:], S0[:], axis=AX.X)
                nc.vector.reduce_max(gm1[:], S1[:], axis=AX.X)
                mb = m_out[:, b:b + 1]
                if g == 0:
                    nc.vector.tensor_tensor(mb, gm0[:], gm1[:], op=ALU.max)
                else:
                    g01 = stat.tile([P, 1], F32, tag="g01", name="g01")
                    nc.vector.tensor_tensor(g01[:], gm0[:], gm1[:], op=ALU.max)
                    nc.vector.tensor_tensor(mb, m_in[:, b:b + 1], g01[:], op=ALU.max)
                negm = stat.tile([P, 1], F32, tag="negm", name="negm")
                nc.vector.tensor_scalar(negm[:], mb, -1.0, None, op0=ALU.mult)
                if g > 0:
                    f = stat.tile([P, 1], F32, tag="f", name="f")
                    nc.scalar.activation(
                        f[:], m_in[:, b:b + 1], AF.Exp, bias=negm[:], scale=1.0
                    )
                    f_h[b] = f
                A = apool.tile([P, GK], F16, tag="A", name=f"A_{g}_{b}")
                ps0 = stat.tile([P, 1], F32, tag="ps0", name="ps0")
                ps1 = stat.tile([P, 1], F32, tag="ps1", name="ps1")
                nc.scalar.activation(
                    A[:, 0:512], S0[:], AF.Exp, bias=negm[:], scale=1.0,
                    accum_out=ps0[:],
                )
                nc.scalar.activation(
                    A[:, 512:1024], S1[:], AF.Exp, bias=negm[:], scale=1.0,
                    accum_out=ps1[:],
                )
                rb_in, rb_out = rs_in[:, b:b + 1], rs_out[:, b:b + 1]
                if g == 0:
                    nc.vector.tensor_tensor(rb_out, ps0[:], ps1[:], op=ALU.add)
                else:
                    pss = stat.tile([P, 1], F32, tag="pss", name="pss")
                    nc.vector.tensor_tensor(pss[:], ps0[:], ps1[:], op=ALU.add)
                    nc.vector.scalar_tensor_tensor(
                        rb_out, rb_in, f_h[b][:], pss[:], op0=ALU.mult, op1=ALU.add
                    )
                AT = atp.tile([P, ND, P], F16, tag="AT", name=f"AT_{g}_{b}")
                for kc in range(8):
                    nc.sync.dma_start(
                        AT[:, kc, :], A[:, kc * P:(kc + 1) * P], transpose=True
                    )
                at_h[b] = AT

            def pv(b):
                po = pop.tile([P, VD], F32, tag="po", name=f"po_{g}_{b}")
                AT = at_h.pop(b)
                for kc in range(8):
                    nc.tensor.matmul(
                        po[:],
                        AT[:, kc, :],
                        Vg[:, kc, :],
                        start=(kc == 0),
                        stop=(kc == 7),
                    )
                ob_out, ob_in = O_out[:, b, :], O_in[:, b, :]
                if g == 0:
                    nc.vector.tensor_copy(ob_out, po[:])
                else:
                    nc.vector.scalar_tensor_tensor(
                        ob_out, ob_in, f_h.pop(b)[:], po[:], op0=ALU.mult, op1=ALU.add
                    )

            for b in range(NB):
                qk(b)
                if b >= DEPTH:
                    pv(b - DEPTH)
            for b in range(NB - DEPTH, NB):
                pv(b)

        # ---- epilogue: out = O * (NORM / rs) + NORM * bv ----
        m_fin, rs_fin, O_fin = m_st[NG % 2], rs_st[NG % 2], O_st[NG % 2]
        del m_fin
        for b in range(NB):
            rinv = stat.tile([P, 1], F32, tag="rinv", name="rinv")
            nc.vector.reciprocal(rinv[:], rs_fin[:, b:b + 1])
            rn = stat.tile([P, 1], F32, tag="rn", name="rn")
            nc.vector.tensor_scalar(rn[:], rinv[:], NORM, None, op0=ALU.mult)
            of = outp.tile([P, VD], F32, tag="of", name=f"of{b}")
            nc.vector.scalar_tensor_tensor(
                of[:], O_fin[:, b, :], rn[:], bvN[:], op0=ALU.mult, op1=ALU.add
            )
            nc.scalar.dma_start(out_ap[b * P:(b + 1) * P, :], of[:])


_cached = {}


def _build():
    if "nc" in _cached:
        return _cached["nc"]
    nc = bass.Bass("TRN2", target_bir_lowering=False, debug=False)
    aps = {
        "qT": nc.dram_tensor("qT", [D, RL], F16, kind="ExternalInput").ap(),
        "kT": nc.dram_tensor("kT", [D, KEYS], F16, kind="ExternalInput").ap(),
        "vT": nc.dram_tensor("vT", [D, KEYS], F16, kind="ExternalInput").ap(),
        "wqT": nc.dram_tensor("wqT", [D, C], F16, kind="ExternalInput").ap(),
        "wkT": nc.dram_tensor("wkT", [D, C], F16, kind="ExternalInput").ap(),
        "wvT": nc.dram_tensor("wvT", [D, C], F16, kind="ExternalInput").ap(),
        "bq": nc.dram_tensor("bq", [C], F32, kind="ExternalInput").ap(),
        "bv": nc.dram_tensor("bv", [VD], F32, kind="ExternalInput").ap(),
        "out": nc.dram_tensor("out", [RL, VD], F32, kind="ExternalOutput").ap(),
    }
    with tile.TileContext(nc) as tc:
        _emit(nc, tc, aps)
    _split_multi_waits(nc)
    _cached["nc"] = nc
    return nc


def kernel(q, k, v, Wq, bq, Wk, bk, Wv, bv, _trace=False, _tmpdir=None):
    del bk  # provably cancels inside the softmax
    nc = _build()

    def f16T(a):
        return np.ascontiguousarray(
            np.asarray(a, dtype=np.float32).astype(np.float16).T
        )

    q16 = np.asarray(q, dtype=np.float32).astype(np.float16)
    base = {
        "kT": f16T(k), "vT": f16T(v),
        "wqT": f16T(Wq), "wkT": f16T(Wk), "wvT": f16T(Wv),
        "bq": np.ascontiguousarray(np.asarray(bq, dtype=np.float32)),
        "bv": np.ascontiguousarray(np.asarray(bv, dtype=np.float32)),
    }
    in_maps = [
        dict(base, qT=np.ascontiguousarray(q16[c * RL:(c + 1) * RL].T))
        for c in range(NCORES)
    ]
    res = bass_utils.run_bass_kernel_spmd(
        nc, in_maps, core_ids=list(range(NCORES)), trace=_trace, tmpdir=_tmpdir
    )
    out = np.concatenate([res.results[c]["out"] for c in range(NCORES)], axis=0)
    if _trace:
        kernel.last_results = res
    return out


# revision 9
# speedup vs baseline: 2.5621x; 1.9701x over previous
"""Cross-attention kernel for Trainium2, sharded over 8 NeuronCores.

Shards query rows across cores (1024 rows each); K/V work is replicated.

Structure (flash-attention streaming, one pass over 8 key-groups of 1024):
  - Host pre-marshals inputs: fp32->fp16 cast + transpose into the exact
    [contraction-on-partition] layouts the PE needs.  The device does zero
    layout work for inputs; all FLOPs (projections + attention) stay on
    device.  Device DMA-in is ~41MB/core instead of the ~150MB a DRAM
    staging round-trip costs.
  - Per group g: load kT/vT slab (gpsimd SWDGE, double-buffered), project
    K^T [c,keys] and V [keys,v] (PE, evicted fp16 by the scalar engine),
    then stream attention for all 8 query row-blocks: S = Q@K^T (PSUM),
    online-softmax rescale (DVE/Act), A^T via PE transpose-mode matmuls
    into one fp16 PSUM bank (bulk-evicted by one DVE copy), and PV
    accumulated into per-block fp32 accumulators (flash rescale by
    exp(m_old - m_new) via one fused scalar_tensor_tensor).
  - The per-block stages are software-pipelined [QK(b), PV(b-2), T(b-1)]
    so the PE never waits for the softmax of the block it just produced.
  - PSUM budget exactly 8 banks: 3 proj/PV + 2x2 scores + 1 A^T.

Algebraic simplifications:
  - bk is dropped: it adds a per-row constant to scores, softmax cancels it.
  - bv is folded into the epilogue: atten rows sum to NORM after scaling,
    so out += NORM * bv.
  - softmax normalization and the post-softmax 1/sqrt(dk) scale fold into
    one per-row multiply at the end.
"""
import sys

sys.path.insert(0, "/opt/trn_rl_repo")

import numpy as np  # noqa: E402
import concourse.bass as bass  # noqa: E402
import concourse.tile as tile  # noqa: E402
from concourse import mybir  # noqa: E402
from concourse import bass_utils  # noqa: E402
from contextlib import ExitStack  # noqa: E402

F16 = mybir.dt.float16
F32 = mybir.dt.float32
AF = mybir.ActivationFunctionType
AX = mybir.AxisListType
ALU = mybir.AluOpType

P = 128
D = 1024             # input dim
ND = D // P          # 8 d-chunks
C = 512              # dim_k
NCC = C // P         # 4 c-chunks
VD = 512             # dim_v
KEYS = 8192
GK = 1024            # keys per group
NG = KEYS // GK      # 8 groups
RL = 1024            # query rows per core
NB = RL // P         # 8 row blocks
NCORES = 8
DEPTH = 3            # QK->PV software pipeline depth
NORM = float(1.0 / np.sqrt(np.float32(C)))

_ws_counter = [0]


def _split_multi_waits(nc):
    """This container's walrus accepts only ONE sync-wait per instruction.
    Move extra waits onto preceding same-engine EventSemaphore insts."""
    for f in nc.m.functions:
        for bb in f.blocks:
            il = bb.instructions
            if not any(
                inst.sync_info is not None and len(inst.sync_info.on_wait or ()) > 1
                for inst in il
            ):
                continue
            new = []
            for inst in il:
                si = inst.sync_info
                if si is not None and len(si.on_wait or ()) > 1:
                    waits = list(si.on_wait)
                    for w in waits[:-1]:
                        _ws_counter[0] += 1
                        new.append(
                            mybir.InstEventSemaphore(
                                name=f"I-ws{_ws_counter[0]}",
                                engine=inst.engine,
                                ins=[],
                                outs=[],
                                sync_info=mybir.SyncInfo(on_wait=[w], on_update=[]),
                            )
                        )
                    del si.on_wait[:-1]
                new.append(inst)
            bb.instructions = new


def _emit(nc, tc, aps):
    qT_r = aps["qT"].rearrange("(dc p) r -> p dc r", p=P)
    kT_r = aps["kT"].rearrange("(dc p) k -> p dc k", p=P)
    vT_r = aps["vT"].rearrange("(dc p) k -> p dc k", p=P)
    out_ap = aps["out"]

    with ExitStack() as top:
        const = top.enter_context(tc.tile_pool(name="const", bufs=1))
        kvin = top.enter_context(tc.tile_pool(name="kvin", bufs=2))
        proj = top.enter_context(tc.tile_pool(name="proj", bufs=2))
        apool = top.enter_context(tc.tile_pool(name="apool", bufs=4))
        atp = top.enter_context(tc.tile_pool(name="atp", bufs=6))
        stat = top.enter_context(tc.tile_pool(name="stat", bufs=6))
        outp = top.enter_context(tc.tile_pool(name="outp", bufs=2))
        pp = top.enter_context(tc.tile_pool(name="pp", bufs=3, space="PSUM"))
        psS = top.enter_context(tc.tile_pool(name="psS", bufs=2, space="PSUM"))
        pat = top.enter_context(tc.tile_pool(name="pat", bufs=1, space="PSUM"))

        # ---- persistent operands ----
        wq = const.tile([P, ND, C], F16, tag="wq")
        wk = const.tile([P, ND, C], F16, tag="wk")
        wv = const.tile([P, ND, C], F16, tag="wv")
        nc.sync.dma_start(wq[:], aps["wqT"].rearrange("(dc p) c -> p dc c", p=P))
        nc.sync.dma_start(wk[:], aps["wkT"].rearrange("(dc p) c -> p dc c", p=P))
        nc.sync.dma_start(wv[:], aps["wvT"].rearrange("(dc p) c -> p dc c", p=P))
        bqT = const.tile([P, NCC], F32, tag="bqT")
        nc.sync.dma_start(bqT[:], aps["bq"].rearrange("(j p) -> p j", p=P))
        bvrow = const.tile([1, VD], F32, tag="bvrow")
        nc.scalar.dma_start(bvrow[:], aps["bv"][None, :])
        ones1 = const.tile([1, P], F32, tag="ones1")
        nc.vector.memset(ones1[:], 1.0)
        ident = const.tile([P, P], F16, tag="ident")
        nc.sync.dma_start(ident[:], aps["ident"][:])

        # bvN[p, v] = NORM * bv[v] broadcast along partitions (rank-1 matmul)
        bvN = const.tile([P, VD], F32, tag="bvN")
        psb0 = pp.tile([P, VD], F32, tag="pp")
        nc.tensor.matmul(psb0[:], ones1[:], bvrow[:], start=True, stop=True)
        nc.scalar.activation(bvN[:], psb0[:], AF.Copy, bias=0.0, scale=NORM)

        # Q^T projection: QT[c-chunk][128, rows] fp16, bias bq folded in.
        QT = const.tile([P, NCC, RL], F16, tag="QT")
        qin = kvin.tile([P, ND, RL], F16, tag="qin", bufs=1)
        nc.sync.dma_start(qin[:], qT_r[:])
        for ci in range(NCC):
            for rh in range(2):
                ps = pp.tile([P, 512], F32, tag="pp")
                for d in range(ND):
                    nc.tensor.matmul(
                        ps[:],
                        wq[:, d, ci * P:(ci + 1) * P],
                        qin[:, d, rh * 512:(rh + 1) * 512],
                        start=(d == 0),
                        stop=(d == ND - 1),
                    )
                nc.scalar.activation(
                    QT[:, ci, rh * 512:(rh + 1) * 512],
                    ps[:],
                    AF.Identity,
                    bias=bqT[:, ci:ci + 1],
                    scale=1.0,
                )

        # flash state, ping-pong by group parity: IN = st[g%2], OUT = st[1-g%2]
        m_st = [const.tile([P, NB], F32, tag=f"m{i}", name=f"m{i}") for i in range(2)]
        rs_st = [const.tile([P, NB], F32, tag=f"rs{i}", name=f"rs{i}") for i in range(2)]
        O_st = [
            const.tile([P, NB, VD], F32, tag=f"O{i}", name=f"O{i}") for i in range(2)
        ]

        def load_group(g):
            kt = kvin.tile([P, ND, GK], F16, tag="kT", name=f"kTg{g}")
            vt = kvin.tile([P, ND, GK], F16, tag="vT", name=f"vTg{g}")
            nc.gpsimd.dma_start(kt[:], kT_r[:, :, g * GK:(g + 1) * GK])
            nc.gpsimd.dma_start(vt[:], vT_r[:, :, g * GK:(g + 1) * GK])
            return kt, vt

        nxt = load_group(0)
        for g in range(NG):
            kt, vt = nxt
            if g + 1 < NG:
                nxt = load_group(g + 1)
            m_in, m_out = m_st[g % 2], m_st[1 - g % 2]
            rs_in, rs_out = rs_st[g % 2], rs_st[1 - g % 2]
            O_in, O_out = O_st[g % 2], O_st[1 - g % 2]

            # ---- K^T / V projections for this group ----
            KT = proj.tile([P, NCC, GK], F16, tag="KT", name=f"KTg{g}")
            Vg = proj.tile([P, ND, VD], F16, tag="V", name=f"Vg{g}")
            for ci in range(NCC):
                for h in range(2):
                    ps = pp.tile([P, 512], F32, tag="pp")
                    for d in range(ND):
                        nc.tensor.matmul(
                            ps[:],
                            wk[:, d, ci * P:(ci + 1) * P],
                            kt[:, d, h * 512:(h + 1) * 512],
                            start=(d == 0),
                            stop=(d == ND - 1),
                        )
                    nc.scalar.copy(KT[:, ci, h * 512:(h + 1) * 512], ps[:])
            for kc in range(8):
                ps = pp.tile([P, 512], F32, tag="pp")
                for d in range(ND):
                    nc.tensor.matmul(
                        ps[:],
                        vt[:, d, kc * P:(kc + 1) * P],
                        wv[:, d, :],
                        start=(d == 0),
                        stop=(d == ND - 1),
                    )
                nc.scalar.copy(Vg[:, kc, :], ps[:])

            # ---- attention over this group, pipelined across row blocks ----
            a_h = {}
            at_h = {}
            f_h = {}

            def qk(b):
                S0 = psS.tile([P, 512], F32, tag="S0", name=f"S0_{g}_{b}")
                S1 = psS.tile([P, 512], F32, tag="S1", name=f"S1_{g}_{b}")
                for S, h in ((S0, 0), (S1, 1)):
                    for ci in range(NCC):
                        nc.tensor.matmul(
                            S[:],
                            QT[:, ci, b * P:(b + 1) * P],
                            KT[:, ci, h * 512:(h + 1) * 512],
                            start=(ci == 0),
                            stop=(ci == NCC - 1),
                        )
                gm0 = stat.tile([P, 1], F32, tag="gm0", name="gm0")
                gm1 = stat.tile([P, 1], F32, tag="gm1", name="gm1")
                nc.vector.reduce_max(gm0[:], S0[:], axis=AX.X)
                nc.vector.reduce_max(gm1[:], S1[:], axis=AX.X)
                mb = m_out[:, b:b + 1]
                if g == 0:
                    nc.vector.tensor_tensor(mb, gm0[:], gm1[:], op=ALU.max)
                else:
                    g01 = stat.tile([P, 1], F32, tag="g01", name="g01")
                    nc.vector.tensor_tensor(g01[:], gm0[:], gm1[:], op=ALU.max)
                    nc.vector.tensor_tensor(mb, m_in[:, b:b + 1], g01[:], op=ALU.max)
                negm = stat.tile([P, 1], F32, tag="negm", name="negm")
                nc.vector.tensor_scalar(negm[:], mb, -1.0, None, op0=ALU.mult)
                if g > 0:
                    f = stat.tile([P, 1], F32, tag="f", name="f")
                    nc.scalar.activation(
                        f[:], m_in[:, b:b + 1], AF.Exp, bias=negm[:], scale=1.0
                    )
                    f_h[b] = f
                A = apool.tile([P, GK], F16, tag="A", name=f"A_{g}_{b}")
                ps0 = stat.tile([P, 1], F32, tag="ps0", name="ps0")
                ps1 = stat.tile([P, 1], F32, tag="ps1", name="ps1")
                nc.scalar.activation(
                    A[:, 0:512], S0[:], AF.Exp, bias=negm[:], scale=1.0,
                    accum_out=ps0[:],
                )
                nc.scalar.activation(
                    A[:, 512:1024], S1[:], AF.Exp, bias=negm[:], scale=1.0,
                    accum_out=ps1[:],
                )
                rb_in, rb_out = rs_in[:, b:b + 1], rs_out[:, b:b + 1]
                if g == 0:
                    nc.vector.tensor_tensor(rb_out, ps0[:], ps1[:], op=ALU.add)
                else:
                    pss = stat.tile([P, 1], F32, tag="pss", name="pss")
                    nc.vector.tensor_tensor(pss[:], ps0[:], ps1[:], op=ALU.add)
                    nc.vector.scalar_tensor_tensor(
                        rb_out, rb_in, f_h[b][:], pss[:], op0=ALU.mult, op1=ALU.add
                    )
                a_h[b] = A

            def tr(b):
                # A^T for all 8 key chunks, PE transpose-mode -> one PSUM bank
                A = a_h.pop(b)
                tp = pat.tile([P, ND, P], F16, tag="tp", name=f"tp_{g}_{b}")
                for kc in range(8):
                    nc.tensor.transpose(
                        tp[:, kc, :], A[:, kc * P:(kc + 1) * P], ident[:]
                    )
                AT = atp.tile([P, ND, P], F16, tag="AT", name=f"AT_{g}_{b}")
                nc.vector.tensor_copy(AT[:], tp[:])
                at_h[b] = AT

            def pv(b):
                po = pp.tile([P, VD], F32, tag="pp", name=f"po_{g}_{b}")
                AT = at_h.pop(b)
                for kc in range(8):
                    nc.tensor.matmul(
                        po[:],
                        AT[:, kc, :],
                        Vg[:, kc, :],
                        start=(kc == 0),
                        stop=(kc == 7),
                    )
                ob_out, ob_in = O_out[:, b, :], O_in[:, b, :]
                if g == 0:
                    nc.vector.tensor_copy(ob_out, po[:])
                else:
                    nc.vector.scalar_tensor_tensor(
                        ob_out, ob_in, f_h.pop(b)[:], po[:], op0=ALU.mult, op1=ALU.add
                    )

            # slot b: [QK(b), PV(b-2), T(b-1)] hides softmax+transpose+evict
            for b in range(NB):
                qk(b)
                if b >= 2:
                    pv(b - 2)
                if b >= 1:
                    tr(b - 1)
            pv(NB - 2)
            tr(NB - 1)
            pv(NB - 1)

        # ---- epilogue: out = O * (NORM / rs) + NORM * bv ----
        m_fin, rs_fin, O_fin = m_st[NG % 2], rs_st[NG % 2], O_st[NG % 2]
        del m_fin
        for b in range(NB):
            rinv = stat.tile([P, 1], F32, tag="rinv", name="rinv")
            nc.vector.reciprocal(rinv[:], rs_fin[:, b:b + 1])
            rn = stat.tile([P, 1], F32, tag="rn", name="rn")
            nc.vector.tensor_scalar(rn[:], rinv[:], NORM, None, op0=ALU.mult)
            of = outp.tile([P, VD], F32, tag="of", name=f"of{b}")
            nc.vector.scalar_tensor_tensor(
                of[:], O_fin[:, b, :], rn[:], bvN[:], op0=ALU.mult, op1=ALU.add
            )
            nc.scalar.dma_start(out_ap[b * P:(b + 1) * P, :], of[:])


_cached = {}


def _build():
    if "nc" in _cached:
        return _cached["nc"]
    nc = bass.Bass("TRN2", target_bir_lowering=False, debug=False)
    aps = {
        "qT": nc.dram_tensor("qT", [D, RL], F16, kind="ExternalInput").ap(),
        "kT": nc.dram_tensor("kT", [D, KEYS], F16, kind="ExternalInput").ap(),
        "vT": nc.dram_tensor("vT", [D, KEYS], F16, kind="ExternalInput").ap(),
        "wqT": nc.dram_tensor("wqT", [D, C], F16, kind="ExternalInput").ap(),
        "wkT": nc.dram_tensor("wkT", [D, C], F16, kind="ExternalInput").ap(),
        "wvT": nc.dram_tensor("wvT", [D, C], F16, kind="ExternalInput").ap(),
        "bq": nc.dram_tensor("bq", [C], F32, kind="ExternalInput").ap(),
        "bv": nc.dram_tensor("bv", [VD], F32, kind="ExternalInput").ap(),
        "ident": nc.dram_tensor("ident", [P, P], F16, kind="ExternalInput").ap(),
        "out": nc.dram_tensor("out", [RL, VD], F32, kind="ExternalOutput").ap(),
    }
    with tile.TileContext(nc) as tc:
        _emit(nc, tc, aps)
    _split_multi_waits(nc)
    _cached["nc"] = nc
    return nc


def kernel(q, k, v, Wq, bq, Wk, bk, Wv, bv, _trace=False, _tmpdir=None):
    del bk  # provably cancels inside the softmax
    nc = _build()

    def f16T(a):
        return np.ascontiguousarray(
            np.asarray(a, dtype=np.float32).astype(np.float16).T
        )

    q16 = np.asarray(q, dtype=np.float32).astype(np.float16)
    base = {
        "kT": f16T(k), "vT": f16T(v),
        "wqT": f16T(Wq), "wkT": f16T(Wk), "wvT": f16T(Wv),
        "bq": np.ascontiguousarray(np.asarray(bq, dtype=np.float32)),
        "bv": np.ascontiguousarray(np.asarray(bv, dtype=np.float32)),
        "ident": np.eye(P, dtype=np.float16),
    }
    in_maps = [
        dict(base, qT=np.ascontiguousarray(q16[c * RL:(c + 1) * RL].T))
        for c in range(NCORES)
    ]
    res = bass_utils.run_bass_kernel_spmd(
        nc, in_maps, core_ids=list(range(NCORES)), trace=_trace, tmpdir=_tmpdir
    )
    out = np.concatenate([res.results[c]["out"] for c in range(NCORES)], axis=0)
    if _trace:
        kernel.last_results = res
    return out


# revision 15
# speedup vs baseline: 2.6483x; 1.0336x over previous
"""Cross-attention kernel for Trainium2, sharded over 8 NeuronCores.

Shards query rows across cores (1024 rows each); K/V work is replicated.

Structure (flash-attention streaming, one pass over 8 key-groups of 1024):
  - Host pre-marshals inputs: fp32->fp16 cast + transpose into the exact
    [contraction-on-partition] layouts the PE needs.  The device does zero
    layout work for inputs; all FLOPs (projections + attention) stay on
    device.  Device DMA-in is ~41MB/core instead of the ~150MB a DRAM
    staging round-trip costs.
  - Per group g: load kT/vT slab (gpsimd SWDGE, double-buffered), project
    K^T [c,keys] and V [keys,v] (PE, evicted fp16 by the scalar engine),
    then stream attention for all 8 query row-blocks: S = Q@K^T (PSUM),
    online-softmax rescale (DVE/Act), A^T via PE transpose-mode matmuls
    into one fp16 PSUM bank (bulk-evicted by one DVE copy), and PV
    accumulated into per-block fp32 accumulators (flash rescale by
    exp(m_old - m_new) via one fused scalar_tensor_tensor).
  - The per-block stages are software-pipelined [QK(b), PV(b-2), T(b-1)]
    so the PE never waits for the softmax of the block it just produced.
  - PSUM budget exactly 8 banks: 3 proj/PV + 2x2 scores + 1 A^T.

Algebraic simplifications:
  - bk is dropped: it adds a per-row constant to scores, softmax cancels it.
  - bv is folded into the epilogue: atten rows sum to NORM after scaling,
    so out += NORM * bv.
  - softmax normalization and the post-softmax 1/sqrt(dk) scale fold into
    one per-row multiply at the end.
"""
import sys

sys.path.insert(0, "/opt/trn_rl_repo")

import numpy as np  # noqa: E402
import concourse.bass as bass  # noqa: E402
import concourse.tile as tile  # noqa: E402
from concourse import mybir  # noqa: E402
from concourse import bass_utils  # noqa: E402
from contextlib import ExitStack  # noqa: E402

F16 = mybir.dt.float16
F32 = mybir.dt.float32
AF = mybir.ActivationFunctionType
AX = mybir.AxisListType
ALU = mybir.AluOpType

P = 128
D = 1024             # input dim
ND = D // P          # 8 d-chunks
C = 512              # dim_k
NCC = C // P         # 4 c-chunks
VD = 512             # dim_v
KEYS = 8192
GK = 1024            # keys per group
NG = KEYS // GK      # 8 groups
RL = 1024            # query rows per core
NB = RL // P         # 8 row blocks
NCORES = 8
DEPTH = 3            # QK->PV software pipeline depth
NORM = float(1.0 / np.sqrt(np.float32(C)))

_ws_counter = [0]


def _split_multi_waits(nc):
    """This container's walrus accepts only ONE sync-wait per instruction.
    Move extra waits onto preceding same-engine EventSemaphore insts."""
    for f in nc.m.functions:
        for bb in f.blocks:
            il = bb.instructions
            if not any(
                inst.sync_info is not None and len(inst.sync_info.on_wait or ()) > 1
                for inst in il
            ):
                continue
            new = []
            for inst in il:
                si = inst.sync_info
                if si is not None and len(si.on_wait or ()) > 1:
                    waits = list(si.on_wait)
                    for w in waits[:-1]:
                        _ws_counter[0] += 1
                        new.append(
                            mybir.InstEventSemaphore(
                                name=f"I-ws{_ws_counter[0]}",
                                engine=inst.engine,
                                ins=[],
                                outs=[],
                                sync_info=mybir.SyncInfo(on_wait=[w], on_update=[]),
                            )
                        )
                    del si.on_wait[:-1]
                new.append(inst)
            bb.instructions = new


def _emit(nc, tc, aps):
    qT_r = aps["qT"].rearrange("(dc p) r -> p dc r", p=P)
    kT_r = aps["kT"].rearrange("(dc p) k -> p dc k", p=P)
    vT_r = aps["vT"].rearrange("(dc p) k -> p dc k", p=P)
    out_ap = aps["out"]

    with ExitStack() as top:
        const = top.enter_context(tc.tile_pool(name="const", bufs=1))
        kvin = top.enter_context(tc.tile_pool(name="kvin", bufs=2))
        proj = top.enter_context(tc.tile_pool(name="proj", bufs=2))
        apool = top.enter_context(tc.tile_pool(name="apool", bufs=4))
        atp = top.enter_context(tc.tile_pool(name="atp", bufs=6))
        stat = top.enter_context(tc.tile_pool(name="stat", bufs=6))
        outp = top.enter_context(tc.tile_pool(name="outp", bufs=2))
        pp = top.enter_context(tc.tile_pool(name="pp", bufs=3, space="PSUM"))
        psS = top.enter_context(tc.tile_pool(name="psS", bufs=2, space="PSUM"))
        pat = top.enter_context(tc.tile_pool(name="pat", bufs=1, space="PSUM"))

        # ---- persistent operands ----
        # sync queue: qT first (gates Q-proj), then weights in use order.
        qin = kvin.tile([P, ND, RL], F16, tag="qin", bufs=1)
        nc.sync.dma_start(qin[:], qT_r[:])
        wq = const.tile([P, ND, C], F16, tag="wq")
        wk = const.tile([P, ND, C], F16, tag="wk")
        wv = const.tile([P, ND, C], F16, tag="wv")
        nc.sync.dma_start(wq[:], aps["wqT"].rearrange("(dc p) c -> p dc c", p=P))
        nc.sync.dma_start(wk[:], aps["wkT"].rearrange("(dc p) c -> p dc c", p=P))
        nc.sync.dma_start(wv[:], aps["wvT"].rearrange("(dc p) c -> p dc c", p=P))
        # scalar queue: small descriptor-bound loads, nothing urgent.
        bvrow = const.tile([1, VD], F32, tag="bvrow")
        nc.scalar.dma_start(bvrow[:], aps["bv"][None, :])
        bqT = const.tile([P, NCC], F32, tag="bqT")
        nc.scalar.dma_start(bqT[:], aps["bqT"][:])
        ident = const.tile([P, P], F16, tag="ident")
        nc.scalar.dma_start(ident[:], aps["ident"][:])
        ones1 = const.tile([1, P], F32, tag="ones1")
        nc.vector.memset(ones1[:], 1.0)

        # bvN[p, v] = NORM * bv[v] broadcast along partitions (rank-1 matmul)
        bvN = const.tile([P, VD], F32, tag="bvN")
        psb0 = pp.tile([P, VD], F32, tag="pp")
        nc.tensor.matmul(psb0[:], ones1[:], bvrow[:], start=True, stop=True)
        nc.scalar.activation(bvN[:], psb0[:], AF.Copy, bias=0.0, scale=NORM)

        # Q^T projection: QT[c-chunk][128, rows] fp16, bias bq folded in.
        QT = const.tile([P, NCC, RL], F16, tag="QT")
        for ci in range(NCC):
            for rh in range(2):
                ps = pp.tile([P, 512], F32, tag="pp")
                for d in range(ND):
                    nc.tensor.matmul(
                        ps[:],
                        wq[:, d, ci * P:(ci + 1) * P],
                        qin[:, d, rh * 512:(rh + 1) * 512],
                        start=(d == 0),
                        stop=(d == ND - 1),
                    )
                nc.scalar.activation(
                    QT[:, ci, rh * 512:(rh + 1) * 512],
                    ps[:],
                    AF.Identity,
                    bias=bqT[:, ci:ci + 1],
                    scale=1.0,
                )

        # flash state, ping-pong by group parity: IN = st[g%2], OUT = st[1-g%2]
        m_st = [const.tile([P, NB], F32, tag=f"m{i}", name=f"m{i}") for i in range(2)]
        rs_st = [const.tile([P, NB], F32, tag=f"rs{i}", name=f"rs{i}") for i in range(2)]
        O_st = [
            const.tile([P, NB, VD], F32, tag=f"O{i}", name=f"O{i}") for i in range(2)
        ]

        def load_group(g):
            kt = kvin.tile([P, ND, GK], F16, tag="kT", name=f"kTg{g}")
            vt = kvin.tile([P, ND, GK], F16, tag="vT", name=f"vTg{g}")
            nc.gpsimd.dma_start(kt[:], kT_r[:, :, g * GK:(g + 1) * GK])
            nc.gpsimd.dma_start(vt[:], vT_r[:, :, g * GK:(g + 1) * GK])
            return kt, vt

        def make_proj(g, kt, vt):
            """K^T / V projection emitters for group g: 16 psum-group closures."""
            KT = proj.tile([P, NCC, GK], F16, tag="KT", name=f"KTg{g}")
            Vg = proj.tile([P, ND, VD], F16, tag="V", name=f"Vg{g}")
            chunks = []

            def k_chunk(ci, h):
                ps = pp.tile([P, 512], F32, tag="pp", name="psk")
                for d in range(ND):
                    nc.tensor.matmul(
                        ps[:],
                        wk[:, d, ci * P:(ci + 1) * P],
                        kt[:, d, h * 512:(h + 1) * 512],
                        start=(d == 0),
                        stop=(d == ND - 1),
                    )
                nc.scalar.copy(KT[:, ci, h * 512:(h + 1) * 512], ps[:])

            def v_chunk(kc):
                ps = pp.tile([P, 512], F32, tag="pp", name="psv")
                for d in range(ND):
                    nc.tensor.matmul(
                        ps[:],
                        vt[:, d, kc * P:(kc + 1) * P],
                        wv[:, d, :],
                        start=(d == 0),
                        stop=(d == ND - 1),
                    )
                nc.scalar.copy(Vg[:, kc, :], ps[:])

            for ci in range(NCC):
                for h in range(2):
                    chunks.append(lambda ci=ci, h=h: k_chunk(ci, h))
            for kc in range(8):
                chunks.append(lambda kc=kc: v_chunk(kc))
            return KT, Vg, chunks

        nxt = load_group(0)
        nxt_proj = make_proj(0, *nxt)
        pending = nxt_proj[2]
        for g in range(NG):
            KT, Vg, _ = nxt_proj
            if g + 1 < NG:
                nxt = load_group(g + 1)
            m_in, m_out = m_st[g % 2], m_st[1 - g % 2]
            rs_in, rs_out = rs_st[g % 2], rs_st[1 - g % 2]
            O_in, O_out = O_st[g % 2], O_st[1 - g % 2]

            # ---- K^T / V projection chunks not already emitted in g-1 tail
            for c in pending:
                c()

            # ---- attention over this group, pipelined across row blocks ----
            a_h = {}
            at_h = {}
            f_h = {}

            def qk(b):
                S0 = psS.tile([P, 512], F32, tag="S0", name=f"S0_{g}_{b}")
                S1 = psS.tile([P, 512], F32, tag="S1", name=f"S1_{g}_{b}")
                for S, h in ((S0, 0), (S1, 1)):
                    for ci in range(NCC):
                        nc.tensor.matmul(
                            S[:],
                            QT[:, ci, b * P:(b + 1) * P],
                            KT[:, ci, h * 512:(h + 1) * 512],
                            start=(ci == 0),
                            stop=(ci == NCC - 1),
                        )
                gm0 = stat.tile([P, 1], F32, tag="gm0", name="gm0")
                gm1 = stat.tile([P, 1], F32, tag="gm1", name="gm1")
                nc.vector.reduce_max(gm0[:], S0[:], axis=AX.X)
                nc.vector.reduce_max(gm1[:], S1[:], axis=AX.X)
                mb = m_out[:, b:b + 1]
                if g == 0:
                    nc.vector.tensor_tensor(mb, gm0[:], gm1[:], op=ALU.max)
                else:
                    g01 = stat.tile([P, 1], F32, tag="g01", name="g01")
                    nc.vector.tensor_tensor(g01[:], gm0[:], gm1[:], op=ALU.max)
                    nc.vector.tensor_tensor(mb, m_in[:, b:b + 1], g01[:], op=ALU.max)
                negm = stat.tile([P, 1], F32, tag="negm", name="negm")
                nc.vector.tensor_scalar(negm[:], mb, -1.0, None, op0=ALU.mult)
                if g > 0:
                    f = stat.tile([P, 1], F32, tag="f", name="f")
                    nc.scalar.activation(
                        f[:], m_in[:, b:b + 1], AF.Exp, bias=negm[:], scale=1.0
                    )
                    f_h[b] = f
                A = apool.tile([P, GK], F16, tag="A", name=f"A_{g}_{b}")
                ps0 = stat.tile([P, 1], F32, tag="ps0", name="ps0")
                ps1 = stat.tile([P, 1], F32, tag="ps1", name="ps1")
                nc.scalar.activation(
                    A[:, 0:512], S0[:], AF.Exp, bias=negm[:], scale=1.0,
                    accum_out=ps0[:],
                )
                nc.scalar.activation(
                    A[:, 512:1024], S1[:], AF.Exp, bias=negm[:], scale=1.0,
                    accum_out=ps1[:],
                )
                rb_in, rb_out = rs_in[:, b:b + 1], rs_out[:, b:b + 1]
                if g == 0:
                    nc.vector.tensor_tensor(rb_out, ps0[:], ps1[:], op=ALU.add)
                else:
                    pss = stat.tile([P, 1], F32, tag="pss", name="pss")
                    nc.vector.tensor_tensor(pss[:], ps0[:], ps1[:], op=ALU.add)
                    nc.vector.scalar_tensor_tensor(
                        rb_out, rb_in, f_h[b][:], pss[:], op0=ALU.mult, op1=ALU.add
                    )
                a_h[b] = A

            def tr(b):
                # A^T for all 8 key chunks, PE transpose-mode -> one PSUM bank
                A = a_h.pop(b)
                tp = pat.tile([P, ND, P], F16, tag="tp", name=f"tp_{g}_{b}")
                for kc in range(8):
                    nc.tensor.transpose(
                        tp[:, kc, :], A[:, kc * P:(kc + 1) * P], ident[:]
                    )
                AT = atp.tile([P, ND, P], F16, tag="AT", name=f"AT_{g}_{b}")
                nc.vector.tensor_copy(AT[:], tp[:])
                at_h[b] = AT

            def pv(b):
                po = pp.tile([P, VD], F32, tag="pp", name=f"po_{g}_{b}")
                AT = at_h.pop(b)
                for kc in range(8):
                    nc.tensor.matmul(
                        po[:],
                        AT[:, kc, :],
                        Vg[:, kc, :],
                        start=(kc == 0),
                        stop=(kc == 7),
                    )
                ob_out, ob_in = O_out[:, b, :], O_in[:, b, :]
                if g == 0:
                    nc.vector.tensor_copy(ob_out, po[:])
                else:
                    nc.vector.scalar_tensor_tensor(
                        ob_out, ob_in, f_h.pop(b)[:], po[:], op0=ALU.mult, op1=ALU.add
                    )

            def ep(b):
                # epilogue: out = O * (NORM / rs) + NORM * bv
                rinv = stat.tile([P, 1], F32, tag="rinv", name="rinv")
                nc.vector.reciprocal(rinv[:], rs_out[:, b:b + 1])
                rn = stat.tile([P, 1], F32, tag="rn", name="rn")
                nc.vector.tensor_scalar(rn[:], rinv[:], NORM, None, op0=ALU.mult)
                of = outp.tile([P, VD], F32, tag="of", name=f"of{b}")
                nc.vector.scalar_tensor_tensor(
                    of[:], O_out[:, b, :], rn[:], bvN[:], op0=ALU.mult, op1=ALU.add
                )
                nc.scalar.dma_start(out_ap[b * P:(b + 1) * P, :], of[:])

            last = g == NG - 1

            def pv_ep(b):
                pv(b)
                if last:
                    ep(b)

            # slot b: [QK(b), PV(b-3), T(b-2)] hides softmax+transpose+evict;
            # group tail interleaves the next group's first proj chunks so the
            # PE has independent work while the last transposes/PVs drain.
            for b in range(NB):
                qk(b)
                if b >= 3:
                    pv_ep(b - 3)
                if b >= 2:
                    tr(b - 2)
            if g + 1 < NG:
                nxt_proj = make_proj(g + 1, *nxt)
                nch = nxt_proj[2]
                pv_ep(NB - 3)
                tr(NB - 2)
                nch[0]()
                pv_ep(NB - 2)
                tr(NB - 1)
                nch[1]()
                nch[2]()
                pv_ep(NB - 1)
                pending = nch[3:]
            else:
                pv_ep(NB - 3)
                tr(NB - 2)
                pv_ep(NB - 2)
                tr(NB - 1)
                pv_ep(NB - 1)


_cached = {}


def _build():
    if "nc" in _cached:
        return _cached["nc"]
    nc = bass.Bass("TRN2", target_bir_lowering=False, debug=False)
    aps = {
        "qT": nc.dram_tensor("qT", [D, RL], F16, kind="ExternalInput").ap(),
        "kT": nc.dram_tensor("kT", [D, KEYS], F16, kind="ExternalInput").ap(),
        "vT": nc.dram_tensor("vT", [D, KEYS], F16, kind="ExternalInput").ap(),
        "wqT": nc.dram_tensor("wqT", [D, C], F16, kind="ExternalInput").ap(),
        "wkT": nc.dram_tensor("wkT", [D, C], F16, kind="ExternalInput").ap(),
        "wvT": nc.dram_tensor("wvT", [D, C], F16, kind="ExternalInput").ap(),
        "bqT": nc.dram_tensor("bqT", [P, NCC], F32, kind="ExternalInput").ap(),
        "bv": nc.dram_tensor("bv", [VD], F32, kind="ExternalInput").ap(),
        "ident": nc.dram_tensor("ident", [P, P], F16, kind="ExternalInput").ap(),
        "out": nc.dram_tensor("out", [RL, VD], F32, kind="ExternalOutput").ap(),
    }
    with tile.TileContext(nc) as tc:
        _emit(nc, tc, aps)
    _split_multi_waits(nc)
    _cached["nc"] = nc
    return nc


def kernel(q, k, v, Wq, bq, Wk, bk, Wv, bv, _trace=False, _tmpdir=None):
    del bk  # provably cancels inside the softmax
    nc = _build()

    def f16T(a):
        return np.ascontiguousarray(
            np.asarray(a, dtype=np.float32).astype(np.float16).T
        )

    q16 = np.asarray(q, dtype=np.float32).astype(np.float16)
    base = {
        "kT": f16T(k), "vT": f16T(v),
        "wqT": f16T(Wq), "wkT": f16T(Wk), "wvT": f16T(Wv),
        "bqT": np.ascontiguousarray(
            np.asarray(bq, dtype=np.float32).reshape(NCC, P).T
        ),
        "bv": np.ascontiguousarray(np.asarray(bv, dtype=np.float32)),
        "ident": np.eye(P, dtype=np.float16),
    }
    in_maps = [
        dict(base, qT=np.ascontiguousarray(q16[c * RL:(c + 1) * RL].T))
        for c in range(NCORES)
    ]
    res = bass_utils.run_bass_kernel_spmd(
        nc, in_maps, core_ids=list(range(NCORES)), trace=_trace, tmpdir=_tmpdir
    )
    out = np.concatenate([res.results[c]["out"] for c in range(NCORES)], axis=0)
    if _trace:
        kernel.last_results = res
    return out


# revision 23
# speedup vs baseline: 2.6512x; 1.0011x over previous
"""Cross-attention kernel for Trainium2, sharded over 8 NeuronCores.

Shards query rows across cores (1024 rows each); K/V work is replicated.

Structure (flash-attention streaming, one pass over 8 key-groups of 1024):
  - Host pre-marshals inputs: fp32->fp16 cast + transpose into the exact
    [contraction-on-partition] layouts the PE needs.  The device does zero
    layout work for inputs; all FLOPs (projections + attention) stay on
    device.  Device DMA-in is ~41MB/core instead of the ~150MB a DRAM
    staging round-trip costs.
  - Per group g: load kT/vT slab (gpsimd SWDGE, double-buffered), project
    K^T [c,keys] and V [keys,v] (PE, evicted fp16 by the scalar engine),
    then stream attention for all 8 query row-blocks: S = Q@K^T (PSUM),
    online-softmax rescale (DVE/Act), A^T via PE transpose-mode matmuls
    into one fp16 PSUM bank (bulk-evicted by one DVE copy), and PV
    accumulated into per-block fp32 accumulators (flash rescale by
    exp(m_old - m_new) via one fused scalar_tensor_tensor).
  - The per-block stages are software-pipelined [QK(b), PV(b-2), T(b-1)]
    so the PE never waits for the softmax of the block it just produced.
  - PSUM budget exactly 8 banks: 3 proj/PV + 2x2 scores + 1 A^T.

Algebraic simplifications:
  - bk is dropped: it adds a per-row constant to scores, softmax cancels it.
  - bv is folded into the epilogue: atten rows sum to NORM after scaling,
    so out += NORM * bv.
  - softmax normalization and the post-softmax 1/sqrt(dk) scale fold into
    one per-row multiply at the end.
"""
import sys

sys.path.insert(0, "/opt/trn_rl_repo")

import numpy as np  # noqa: E402
import concourse.bass as bass  # noqa: E402
import concourse.tile as tile  # noqa: E402
from concourse import mybir  # noqa: E402
from concourse import bass_utils  # noqa: E402
from contextlib import ExitStack  # noqa: E402

F16 = mybir.dt.float16
F32 = mybir.dt.float32
AF = mybir.ActivationFunctionType
AX = mybir.AxisListType
ALU = mybir.AluOpType

P = 128
D = 1024             # input dim
ND = D // P          # 8 d-chunks
C = 512              # dim_k
NCC = C // P         # 4 c-chunks
VD = 512             # dim_v
KEYS = 8192
GK = 1024            # keys per group
NG = KEYS // GK      # 8 groups
RL = 1024            # query rows per core
NB = RL // P         # 8 row blocks
NCORES = 8
DEPTH = 3            # QK->PV software pipeline depth
NORM = float(1.0 / np.sqrt(np.float32(C)))

_ws_counter = [0]


def _split_multi_waits(nc):
    """This container's walrus accepts only ONE sync-wait per instruction.
    Move extra waits onto preceding same-engine EventSemaphore insts."""
    for f in nc.m.functions:
        for bb in f.blocks:
            il = bb.instructions
            if not any(
                inst.sync_info is not None and len(inst.sync_info.on_wait or ()) > 1
                for inst in il
            ):
                continue
            new = []
            for inst in il:
                si = inst.sync_info
                if si is not None and len(si.on_wait or ()) > 1:
                    waits = list(si.on_wait)
                    for w in waits[:-1]:
                        _ws_counter[0] += 1
                        new.append(
                            mybir.InstEventSemaphore(
                                name=f"I-ws{_ws_counter[0]}",
                                engine=inst.engine,
                                ins=[],
                                outs=[],
                                sync_info=mybir.SyncInfo(on_wait=[w], on_update=[]),
                            )
                        )
                    del si.on_wait[:-1]
                new.append(inst)
            bb.instructions = new


def _emit(nc, tc, aps):
    qT_r = aps["qT"]
    kT_r = aps["kT"]
    vT_r = aps["vT"]
    out_ap = aps["out"]

    with ExitStack() as top:
        const = top.enter_context(tc.tile_pool(name="const", bufs=1))
        kvin = top.enter_context(tc.tile_pool(name="kvin", bufs=2))
        proj = top.enter_context(tc.tile_pool(name="proj", bufs=2))
        apool = top.enter_context(tc.tile_pool(name="apool", bufs=4))
        atp = top.enter_context(tc.tile_pool(name="atp", bufs=6))
        stat = top.enter_context(tc.tile_pool(name="stat", bufs=6))
        outp = top.enter_context(tc.tile_pool(name="outp", bufs=2))
        pp = top.enter_context(tc.tile_pool(name="pp", bufs=3, space="PSUM"))
        psS = top.enter_context(tc.tile_pool(name="psS", bufs=2, space="PSUM"))
        pat = top.enter_context(tc.tile_pool(name="pat", bufs=1, space="PSUM"))

        # ---- persistent operands ----
        # sync queue: qT first (gates Q-proj); scalar queue: wq first.
        qin = kvin.tile([P, ND, RL], F16, tag="qin", bufs=1)
        nc.sync.dma_start(qin[:], qT_r[:])
        wq = const.tile([P, ND, C], F16, tag="wq")
        wk = const.tile([P, ND, C], F16, tag="wk")
        wv = const.tile([P, ND, C], F16, tag="wv")
        nc.scalar.dma_start(wq[:], aps["wqT"][:])
        nc.sync.dma_start(wk[:], aps["wkT"][:])
        nc.sync.dma_start(wv[:], aps["wvT"][:])
        bvrow = const.tile([1, VD], F32, tag="bvrow")
        nc.sync.dma_start(bvrow[:], aps["bv"][None, :])
        bqT = const.tile([P, NCC], F32, tag="bqT")
        nc.scalar.dma_start(bqT[:], aps["bqT"][:])
        ident = const.tile([P, P], F16, tag="ident")
        nc.scalar.dma_start(ident[:], aps["ident"][:])
        ones1 = const.tile([1, P], F32, tag="ones1")
        nc.vector.memset(ones1[:], 1.0)

        # Q^T projection: QT[c-chunk][128, rows] fp16, bias bq folded in.
        QT = const.tile([P, NCC, RL], F16, tag="QT")
        for ci in range(NCC):
            for rh in range(2):
                ps = pp.tile([P, 512], F32, tag="pp")
                for d in range(ND):
                    nc.tensor.matmul(
                        ps[:],
                        wq[:, d, ci * P:(ci + 1) * P],
                        qin[:, d, rh * 512:(rh + 1) * 512],
                        start=(d == 0),
                        stop=(d == ND - 1),
                    )
                nc.scalar.activation(
                    QT[:, ci, rh * 512:(rh + 1) * 512],
                    ps[:],
                    AF.Identity,
                    bias=bqT[:, ci:ci + 1],
                    scale=1.0,
                )

        # bvN[p, v] = NORM * bv[v] broadcast along partitions (rank-1 matmul)
        bvN = const.tile([P, VD], F32, tag="bvN")
        psb0 = pp.tile([P, VD], F32, tag="pp")
        nc.tensor.matmul(psb0[:], ones1[:], bvrow[:], start=True, stop=True)
        nc.scalar.activation(bvN[:], psb0[:], AF.Copy, bias=0.0, scale=NORM)

        # flash state, ping-pong by group parity: IN = st[g%2], OUT = st[1-g%2]
        m_st = [const.tile([P, NB], F32, tag=f"m{i}", name=f"m{i}") for i in range(2)]
        rs_st = [const.tile([P, NB], F32, tag=f"rs{i}", name=f"rs{i}") for i in range(2)]
        O_st = [
            const.tile([P, NB, VD], F32, tag=f"O{i}", name=f"O{i}") for i in range(2)
        ]

        def load_group(g):
            kt = kvin.tile([P, ND, GK], F16, tag="kT", name=f"kTg{g}")
            vt = kvin.tile([P, ND, GK], F16, tag="vT", name=f"vTg{g}")
            nc.gpsimd.dma_start(kt[:], kT_r[:, g, :, :])
            nc.gpsimd.dma_start(vt[:], vT_r[:, g, :, :])
            return kt, vt

        def make_proj(g, kt, vt):
            """K^T / V projection emitters for group g: 16 psum-group closures."""
            KT = proj.tile([P, NCC, GK], F16, tag="KT", name=f"KTg{g}")
            Vg = proj.tile([P, ND, VD], F16, tag="V", name=f"Vg{g}")
            chunks = []

            def k_chunk(ci, h):
                ps = pp.tile([P, 512], F32, tag="pp", name="psk")
                for d in range(ND):
                    nc.tensor.matmul(
                        ps[:],
                        wk[:, d, ci * P:(ci + 1) * P],
                        kt[:, d, h * 512:(h + 1) * 512],
                        start=(d == 0),
                        stop=(d == ND - 1),
                    )
                nc.scalar.copy(KT[:, ci, h * 512:(h + 1) * 512], ps[:])

            def v_chunk(kc):
                ps = pp.tile([P, 512], F32, tag="pp", name="psv")
                for d in range(ND):
                    nc.tensor.matmul(
                        ps[:],
                        vt[:, d, kc * P:(kc + 1) * P],
                        wv[:, d, :],
                        start=(d == 0),
                        stop=(d == ND - 1),
                    )
                nc.scalar.copy(Vg[:, kc, :], ps[:])

            for ci in range(NCC):
                for h in range(2):
                    chunks.append(lambda ci=ci, h=h: k_chunk(ci, h))
            for kc in range(8):
                chunks.append(lambda kc=kc: v_chunk(kc))
            return KT, Vg, chunks

        nxt = load_group(0)
        nxt_proj = make_proj(0, *nxt)
        pending = nxt_proj[2]
        for g in range(NG):
            KT, Vg, _ = nxt_proj
            if g + 1 < NG:
                nxt = load_group(g + 1)
            m_in, m_out = m_st[g % 2], m_st[1 - g % 2]
            rs_in, rs_out = rs_st[g % 2], rs_st[1 - g % 2]
            O_in, O_out = O_st[g % 2], O_st[1 - g % 2]

            # K^T / V projection chunks not emitted in g-1's tail: bulk now,
            # save the last three as PE filler for slots 0-2 (whose softmax
            # latency has no PV/T work to hide behind yet).
            for c in pending[:-3]:
                c()
            slot_fill = pending[-3:]

            # ---- attention over this group, pipelined across row blocks ----
            a_h = {}
            at_h = {}
            f_h = {}

            def qk(b):
                S0 = psS.tile([P, 512], F32, tag="S0", name=f"S0_{g}_{b}")
                S1 = psS.tile([P, 512], F32, tag="S1", name=f"S1_{g}_{b}")
                for S, h in ((S0, 0), (S1, 1)):
                    for ci in range(NCC):
                        nc.tensor.matmul(
                            S[:],
                            QT[:, ci, b * P:(b + 1) * P],
                            KT[:, ci, h * 512:(h + 1) * 512],
                            start=(ci == 0),
                            stop=(ci == NCC - 1),
                        )
                gm0 = stat.tile([P, 1], F32, tag="gm0", name="gm0")
                gm1 = stat.tile([P, 1], F32, tag="gm1", name="gm1")
                nc.vector.reduce_max(gm0[:], S0[:], axis=AX.X)
                nc.vector.reduce_max(gm1[:], S1[:], axis=AX.X)
                mb = m_out[:, b:b + 1]
                if g == 0:
                    nc.vector.tensor_tensor(mb, gm0[:], gm1[:], op=ALU.max)
                else:
                    g01 = stat.tile([P, 1], F32, tag="g01", name="g01")
                    nc.vector.tensor_tensor(g01[:], gm0[:], gm1[:], op=ALU.max)
                    nc.vector.tensor_tensor(mb, m_in[:, b:b + 1], g01[:], op=ALU.max)
                negm = stat.tile([P, 1], F32, tag="negm", name="negm")
                nc.vector.tensor_scalar(negm[:], mb, -1.0, None, op0=ALU.mult)
                if g > 0:
                    f = stat.tile([P, 1], F32, tag="f", name="f")
                    nc.scalar.activation(
                        f[:], m_in[:, b:b + 1], AF.Exp, bias=negm[:], scale=1.0
                    )
                    f_h[b] = f
                A = apool.tile([P, GK], F16, tag="A", name=f"A_{g}_{b}")
                ps0 = stat.tile([P, 1], F32, tag="ps0", name="ps0")
                ps1 = stat.tile([P, 1], F32, tag="ps1", name="ps1")
                nc.scalar.activation(
                    A[:, 0:512], S0[:], AF.Exp, bias=negm[:], scale=1.0,
                    accum_out=ps0[:],
                )
                nc.scalar.activation(
                    A[:, 512:1024], S1[:], AF.Exp, bias=negm[:], scale=1.0,
                    accum_out=ps1[:],
                )
                rb_in, rb_out = rs_in[:, b:b + 1], rs_out[:, b:b + 1]
                if g == 0:
                    nc.vector.tensor_tensor(rb_out, ps0[:], ps1[:], op=ALU.add)
                else:
                    pss = stat.tile([P, 1], F32, tag="pss", name="pss")
                    nc.vector.tensor_tensor(pss[:], ps0[:], ps1[:], op=ALU.add)
                    nc.vector.scalar_tensor_tensor(
                        rb_out, rb_in, f_h[b][:], pss[:], op0=ALU.mult, op1=ALU.add
                    )
                a_h[b] = A

            def tr(b):
                # A^T for all 8 key chunks, PE transpose-mode -> one PSUM bank
                A = a_h.pop(b)
                tp = pat.tile([P, ND, P], F16, tag="tp", name=f"tp_{g}_{b}")
                for kc in range(8):
                    nc.tensor.transpose(
                        tp[:, kc, :], A[:, kc * P:(kc + 1) * P], ident[:]
                    )
                AT = atp.tile([P, ND, P], F16, tag="AT", name=f"AT_{g}_{b}")
                nc.scalar.copy(AT[:], tp[:])
                at_h[b] = AT

            def pv(b):
                po = pp.tile([P, VD], F32, tag="pp", name=f"po_{g}_{b}")
                AT = at_h.pop(b)
                for kc in range(8):
                    nc.tensor.matmul(
                        po[:],
                        AT[:, kc, :],
                        Vg[:, kc, :],
                        start=(kc == 0),
                        stop=(kc == 7),
                    )
                ob_out, ob_in = O_out[:, b, :], O_in[:, b, :]
                if g == 0:
                    nc.vector.tensor_copy(ob_out, po[:])
                else:
                    nc.vector.scalar_tensor_tensor(
                        ob_out, ob_in, f_h.pop(b)[:], po[:], op0=ALU.mult, op1=ALU.add
                    )

            def ep(b):
                # epilogue: out = O * (NORM / rs) + NORM * bv
                rinv = stat.tile([P, 1], F32, tag="rinv", name="rinv")
                nc.vector.reciprocal(rinv[:], rs_out[:, b:b + 1])
                rn = stat.tile([P, 1], F32, tag="rn", name="rn")
                nc.vector.tensor_scalar(rn[:], rinv[:], NORM, None, op0=ALU.mult)
                of = outp.tile([P, VD], F32, tag="of", name=f"of{b}")
                nc.vector.scalar_tensor_tensor(
                    of[:], O_out[:, b, :], rn[:], bvN[:], op0=ALU.mult, op1=ALU.add
                )
                nc.scalar.dma_start(out_ap[b * P:(b + 1) * P, :], of[:])

            last = g == NG - 1

            def pv_ep(b):
                pv(b)
                if last:
                    ep(b)

            # slot b: [QK(b), PV(b-3), T(b-2)] hides softmax+transpose+evict;
            # slots 0-2 use leftover proj chunks as filler, the group tail
            # interleaves the next group's first proj chunks so the PE has
            # independent work while the last transposes/PVs drain.
            for b in range(NB):
                qk(b)
                if b < 3:
                    slot_fill[b]()
                if b >= 3:
                    pv_ep(b - 3)
                if b >= 2:
                    tr(b - 2)
            if g + 1 < NG:
                nxt_proj = make_proj(g + 1, *nxt)
                nch = nxt_proj[2]
                pv_ep(NB - 3)
                tr(NB - 2)
                nch[0]()
                pv_ep(NB - 2)
                tr(NB - 1)
                nch[1]()
                pv_ep(NB - 1)
                nch[2]()
                pending = nch[3:]
            else:
                pv_ep(NB - 3)
                tr(NB - 2)
                pv_ep(NB - 2)
                tr(NB - 1)
                pv_ep(NB - 1)


_cached = {}


def _build():
    if "nc" in _cached:
        return _cached["nc"]
    nc = bass.Bass("TRN2", target_bir_lowering=False, debug=False)
    aps = {
        "qT": nc.dram_tensor("qT", [P, ND, RL], F16, kind="ExternalInput").ap(),
        "kT": nc.dram_tensor("kT", [P, NG, ND, GK], F16, kind="ExternalInput").ap(),
        "vT": nc.dram_tensor("vT", [P, NG, ND, GK], F16, kind="ExternalInput").ap(),
        "wqT": nc.dram_tensor("wqT", [P, ND, C], F16, kind="ExternalInput").ap(),
        "wkT": nc.dram_tensor("wkT", [P, ND, C], F16, kind="ExternalInput").ap(),
        "wvT": nc.dram_tensor("wvT", [P, ND, C], F16, kind="ExternalInput").ap(),
        "bqT": nc.dram_tensor("bqT", [P, NCC], F32, kind="ExternalInput").ap(),
        "bv": nc.dram_tensor("bv", [VD], F32, kind="ExternalInput").ap(),
        "ident": nc.dram_tensor("ident", [P, P], F16, kind="ExternalInput").ap(),
        "out": nc.dram_tensor("out", [RL, VD], F32, kind="ExternalOutput").ap(),
    }
    with tile.TileContext(nc) as tc:
        _emit(nc, tc, aps)
    _split_multi_waits(nc)
    _cached["nc"] = nc
    return nc


def kernel(q, k, v, Wq, bq, Wk, bk, Wv, bv, _trace=False, _tmpdir=None):
    del bk  # provably cancels inside the softmax
    nc = _build()

    def f16(a):
        return np.asarray(a, dtype=np.float32).astype(np.float16)

    def kv_prep(a):
        # [keys, d] -> [p, group, d-chunk, key-in-group]; 16KB/partition rows
        return np.ascontiguousarray(
            f16(a).reshape(NG, GK, ND, P).transpose(3, 0, 2, 1)
        )

    def w_prep(a):
        # [c, d] -> [p, d-chunk, c]
        return np.ascontiguousarray(f16(a).reshape(C, ND, P).transpose(2, 1, 0))

    q16 = f16(q)
    base = {
        "kT": kv_prep(k), "vT": kv_prep(v),
        "wqT": w_prep(Wq), "wkT": w_prep(Wk), "wvT": w_prep(Wv),
        "bqT": np.ascontiguousarray(
            np.asarray(bq, dtype=np.float32).reshape(NCC, P).T
        ),
        "bv": np.ascontiguousarray(np.asarray(bv, dtype=np.float32)),
        "ident": np.eye(P, dtype=np.float16),
    }
    in_maps = [
        dict(
            base,
            qT=np.ascontiguousarray(
                q16[c * RL:(c + 1) * RL].reshape(RL, ND, P).transpose(2, 1, 0)
            ),
        )
        for c in range(NCORES)
    ]
    res = bass_utils.run_bass_kernel_spmd(
        nc, in_maps, core_ids=list(range(NCORES)), trace=_trace, tmpdir=_tmpdir
    )
    out = np.concatenate([res.results[c]["out"] for c in range(NCORES)], axis=0)
    if _trace:
        kernel.last_results = res
    return out


# revision 27
# speedup vs baseline: 2.7089x; 1.0218x over previous
"""Cross-attention kernel for Trainium2, sharded over 8 NeuronCores.

Shards query rows across cores (1024 rows each); K/V work is replicated.

Structure (flash-attention streaming, one pass over 8 key-groups of 1024):
  - Host pre-marshals inputs: fp32->fp16 cast + transpose into the exact
    [contraction-on-partition] layouts the PE needs.  The device does zero
    layout work for inputs; all FLOPs (projections + attention) stay on
    device.  Device DMA-in is ~41MB/core instead of the ~150MB a DRAM
    staging round-trip costs.
  - Per group g: load kT/vT slab (gpsimd SWDGE, double-buffered), project
    K^T [c,keys] and V [keys,v] (PE, evicted fp16 by the scalar engine),
    then stream attention for all 8 query row-blocks: S = Q@K^T (PSUM),
    online-softmax rescale (DVE/Act), A^T via PE transpose-mode matmuls
    into one fp16 PSUM bank (bulk-evicted by one DVE copy), and PV
    accumulated into per-block fp32 accumulators (flash rescale by
    exp(m_old - m_new) via one fused scalar_tensor_tensor).
  - The per-block stages are software-pipelined [QK(b), PV(b-2), T(b-1)]
    so the PE never waits for the softmax of the block it just produced.
  - PSUM budget exactly 8 banks: 3 proj/PV + 2x2 scores + 1 A^T.

Algebraic simplifications:
  - bk is dropped: it adds a per-row constant to scores, softmax cancels it.
  - bv is folded into the epilogue: atten rows sum to NORM after scaling,
    so out += NORM * bv.
  - softmax normalization and the post-softmax 1/sqrt(dk) scale fold into
    one per-row multiply at the end.
"""
import sys

sys.path.insert(0, "/opt/trn_rl_repo")

import numpy as np  # noqa: E402
import concourse.bass as bass  # noqa: E402
import concourse.tile as tile  # noqa: E402
from concourse import mybir  # noqa: E402
from concourse import bass_utils  # noqa: E402
from contextlib import ExitStack  # noqa: E402

F16 = mybir.dt.float16
F32 = mybir.dt.float32
AF = mybir.ActivationFunctionType
AX = mybir.AxisListType
ALU = mybir.AluOpType

P = 128
D = 1024             # input dim
ND = D // P          # 8 d-chunks
C = 512              # dim_k
NCC = C // P         # 4 c-chunks
VD = 512             # dim_v
KEYS = 8192
GK = 1024            # keys per group
NG = KEYS // GK      # 8 groups
RL = 1024            # query rows per core
NB = RL // P         # 8 row blocks
NCORES = 8
DEPTH = 3            # QK->PV software pipeline depth
NORM = float(1.0 / np.sqrt(np.float32(C)))

_ws_counter = [0]


def _split_multi_waits(nc):
    """This container's walrus accepts only ONE sync-wait per instruction.
    Move extra waits onto preceding same-engine EventSemaphore insts."""
    for f in nc.m.functions:
        for bb in f.blocks:
            il = bb.instructions
            if not any(
                inst.sync_info is not None and len(inst.sync_info.on_wait or ()) > 1
                for inst in il
            ):
                continue
            new = []
            for inst in il:
                si = inst.sync_info
                if si is not None and len(si.on_wait or ()) > 1:
                    waits = list(si.on_wait)
                    for w in waits[:-1]:
                        _ws_counter[0] += 1
                        new.append(
                            mybir.InstEventSemaphore(
                                name=f"I-ws{_ws_counter[0]}",
                                engine=inst.engine,
                                ins=[],
                                outs=[],
                                sync_info=mybir.SyncInfo(on_wait=[w], on_update=[]),
                            )
                        )
                    del si.on_wait[:-1]
                new.append(inst)
            bb.instructions = new


def _emit(nc, tc, aps):
    qT_r = aps["qT"]
    kT_r = aps["kT"]
    vT_r = aps["vT"]
    out_ap = aps["out"]

    with ExitStack() as top:
        const = top.enter_context(tc.tile_pool(name="const", bufs=1))
        kvin = top.enter_context(tc.tile_pool(name="kvin", bufs=2))
        proj = top.enter_context(tc.tile_pool(name="proj", bufs=2))
        apool = top.enter_context(tc.tile_pool(name="apool", bufs=4))
        atp = top.enter_context(tc.tile_pool(name="atp", bufs=6))
        stat = top.enter_context(tc.tile_pool(name="stat", bufs=6))
        outp = top.enter_context(tc.tile_pool(name="outp", bufs=2))
        pp = top.enter_context(tc.tile_pool(name="pp", bufs=3, space="PSUM"))
        psS = top.enter_context(tc.tile_pool(name="psS", bufs=2, space="PSUM"))
        pat = top.enter_context(tc.tile_pool(name="pat", bufs=1, space="PSUM"))

        # ---- persistent operands ----
        # sync queue: wq + qT halves first (they gate Q-proj), then wk/wv.
        wq = const.tile([P, ND, C], F16, tag="wq")
        wk = const.tile([P, ND, C], F16, tag="wk")
        wv = const.tile([P, ND, C], F16, tag="wv")
        nc.sync.dma_start(wq[:], aps["wqT"][:])
        qin = kvin.tile([P, 2, ND, 512], F16, tag="qin", bufs=1)
        nc.sync.dma_start(qin[:, 0], qT_r[:, 0])
        nc.sync.dma_start(qin[:, 1], qT_r[:, 1])
        nc.sync.dma_start(wk[:], aps["wkT"][:])
        nc.sync.dma_start(wv[:], aps["wvT"][:])
        bvrow = const.tile([1, VD], F32, tag="bvrow")
        nc.sync.dma_start(bvrow[:], aps["bv"][None, :])
        bqT = const.tile([P, NCC], F32, tag="bqT")
        nc.scalar.dma_start(bqT[:], aps["bqT"][:])
        ident = const.tile([P, P], F16, tag="ident")
        nc.scalar.dma_start(ident[:], aps["ident"][:])
        ones1 = const.tile([1, P], F32, tag="ones1")
        nc.vector.memset(ones1[:], 1.0)

        # Q^T projection: QT[c-chunk][128, rows] fp16, bias bq folded in.
        QT = const.tile([P, NCC, RL], F16, tag="QT")
        for rh in range(2):
            for ci in range(NCC):
                ps = pp.tile([P, 512], F32, tag="pp")
                for d in range(ND):
                    nc.tensor.matmul(
                        ps[:],
                        wq[:, d, ci * P:(ci + 1) * P],
                        qin[:, rh, d, :],
                        start=(d == 0),
                        stop=(d == ND - 1),
                    )
                nc.scalar.activation(
                    QT[:, ci, rh * 512:(rh + 1) * 512],
                    ps[:],
                    AF.Identity,
                    bias=bqT[:, ci:ci + 1],
                    scale=1.0,
                )

        # bvN[p, v] = NORM * bv[v] broadcast along partitions (rank-1 matmul)
        bvN = const.tile([P, VD], F32, tag="bvN")
        psb0 = pp.tile([P, VD], F32, tag="pp")
        nc.tensor.matmul(psb0[:], ones1[:], bvrow[:], start=True, stop=True)
        nc.scalar.activation(bvN[:], psb0[:], AF.Copy, bias=0.0, scale=NORM)

        # flash state, ping-pong by group parity: IN = st[g%2], OUT = st[1-g%2]
        m_st = [const.tile([P, NB], F32, tag=f"m{i}", name=f"m{i}") for i in range(2)]
        rs_st = [const.tile([P, NB], F32, tag=f"rs{i}", name=f"rs{i}") for i in range(2)]
        O_st = [
            const.tile([P, NB, VD], F32, tag=f"O{i}", name=f"O{i}") for i in range(2)
        ]

        def load_group(g):
            kt = kvin.tile([P, ND, GK], F16, tag="kT", name=f"kTg{g}")
            vt = kvin.tile([P, ND, GK], F16, tag="vT", name=f"vTg{g}")
            nc.gpsimd.dma_start(kt[:], kT_r[:, g, :, :])
            nc.gpsimd.dma_start(vt[:], vT_r[:, g, :, :])
            return kt, vt

        def make_proj(g, kt, vt):
            """K^T / V projection emitters for group g: 16 psum-group closures."""
            KT = proj.tile([P, NCC, GK], F16, tag="KT", name=f"KTg{g}")
            Vg = proj.tile([P, ND, VD], F16, tag="V", name=f"Vg{g}")
            chunks = []

            def k_chunk(ci, h):
                ps = pp.tile([P, 512], F32, tag="pp", name="psk")
                for d in range(ND):
                    nc.tensor.matmul(
                        ps[:],
                        wk[:, d, ci * P:(ci + 1) * P],
                        kt[:, d, h * 512:(h + 1) * 512],
                        start=(d == 0),
                        stop=(d == ND - 1),
                    )
                nc.scalar.copy(KT[:, ci, h * 512:(h + 1) * 512], ps[:])

            def v_chunk(kc):
                ps = pp.tile([P, 512], F32, tag="pp", name="psv")
                for d in range(ND):
                    nc.tensor.matmul(
                        ps[:],
                        vt[:, d, kc * P:(kc + 1) * P],
                        wv[:, d, :],
                        start=(d == 0),
                        stop=(d == ND - 1),
                    )
                nc.scalar.copy(Vg[:, kc, :], ps[:])

            for ci in range(NCC):
                for h in range(2):
                    chunks.append(lambda ci=ci, h=h: k_chunk(ci, h))
            for kc in range(8):
                chunks.append(lambda kc=kc: v_chunk(kc))
            return KT, Vg, chunks

        nxt = load_group(0)
        nxt_proj = make_proj(0, *nxt)
        pending = nxt_proj[2]
        for g in range(NG):
            KT, Vg, _ = nxt_proj
            if g + 1 < NG:
                nxt = load_group(g + 1)
            m_in, m_out = m_st[g % 2], m_st[1 - g % 2]
            rs_in, rs_out = rs_st[g % 2], rs_st[1 - g % 2]
            O_in, O_out = O_st[g % 2], O_st[1 - g % 2]

            # K^T / V projection chunks not emitted in g-1's tail: bulk now,
            # save the last three as PE filler for slots 0-2 (whose softmax
            # latency has no PV/T work to hide behind yet).
            for c in pending[:-3]:
                c()
            slot_fill = pending[-3:]

            # ---- attention over this group, pipelined across row blocks ----
            a_h = {}
            at_h = {}
            f_h = {}

            def qk(b):
                S0 = psS.tile([P, 512], F32, tag="S0", name=f"S0_{g}_{b}")
                S1 = psS.tile([P, 512], F32, tag="S1", name=f"S1_{g}_{b}")
                for S, h in ((S0, 0), (S1, 1)):
                    for ci in range(NCC):
                        nc.tensor.matmul(
                            S[:],
                            QT[:, ci, b * P:(b + 1) * P],
                            KT[:, ci, h * 512:(h + 1) * 512],
                            start=(ci == 0),
                            stop=(ci == NCC - 1),
                        )
                gm0 = stat.tile([P, 1], F32, tag="gm0", name="gm0")
                gm1 = stat.tile([P, 1], F32, tag="gm1", name="gm1")
                nc.vector.reduce_max(gm0[:], S0[:], axis=AX.X)
                nc.vector.reduce_max(gm1[:], S1[:], axis=AX.X)
                mb = m_out[:, b:b + 1]
                if g == 0:
                    nc.vector.tensor_tensor(mb, gm0[:], gm1[:], op=ALU.max)
                else:
                    g01 = stat.tile([P, 1], F32, tag="g01", name="g01")
                    nc.vector.tensor_tensor(g01[:], gm0[:], gm1[:], op=ALU.max)
                    nc.vector.tensor_tensor(mb, m_in[:, b:b + 1], g01[:], op=ALU.max)
                negm = stat.tile([P, 1], F32, tag="negm", name="negm")
                nc.vector.tensor_scalar(negm[:], mb, -1.0, None, op0=ALU.mult)
                if g > 0:
                    f = stat.tile([P, 1], F32, tag="f", name="f")
                    nc.scalar.activation(
                        f[:], m_in[:, b:b + 1], AF.Exp, bias=negm[:], scale=1.0
                    )
                    f_h[b] = f
                A = apool.tile([P, GK], F16, tag="A", name=f"A_{g}_{b}")
                ps0 = stat.tile([P, 1], F32, tag="ps0", name="ps0")
                ps1 = stat.tile([P, 1], F32, tag="ps1", name="ps1")
                nc.scalar.activation(
                    A[:, 0:512], S0[:], AF.Exp, bias=negm[:], scale=1.0,
                    accum_out=ps0[:],
                )
                nc.scalar.activation(
                    A[:, 512:1024], S1[:], AF.Exp, bias=negm[:], scale=1.0,
                    accum_out=ps1[:],
                )
                rb_in, rb_out = rs_in[:, b:b + 1], rs_out[:, b:b + 1]
                if g == 0:
                    nc.vector.tensor_tensor(rb_out, ps0[:], ps1[:], op=ALU.add)
                else:
                    pss = stat.tile([P, 1], F32, tag="pss", name="pss")
                    nc.vector.tensor_tensor(pss[:], ps0[:], ps1[:], op=ALU.add)
                    nc.vector.scalar_tensor_tensor(
                        rb_out, rb_in, f_h[b][:], pss[:], op0=ALU.mult, op1=ALU.add
                    )
                a_h[b] = A

            def tr(b):
                # A^T for all 8 key chunks, PE transpose-mode -> one PSUM bank
                A = a_h.pop(b)
                tp = pat.tile([P, ND, P], F16, tag="tp", name=f"tp_{g}_{b}")
                for kc in range(8):
                    nc.tensor.transpose(
                        tp[:, kc, :], A[:, kc * P:(kc + 1) * P], ident[:]
                    )
                AT = atp.tile([P, ND, P], F16, tag="AT", name=f"AT_{g}_{b}")
                nc.scalar.copy(AT[:], tp[:])
                at_h[b] = AT

            def pv(b):
                po = pp.tile([P, VD], F32, tag="pp", name=f"po_{g}_{b}")
                AT = at_h.pop(b)
                for kc in range(8):
                    nc.tensor.matmul(
                        po[:],
                        AT[:, kc, :],
                        Vg[:, kc, :],
                        start=(kc == 0),
                        stop=(kc == 7),
                    )
                ob_out, ob_in = O_out[:, b, :], O_in[:, b, :]
                if g == 0:
                    nc.vector.tensor_copy(ob_out, po[:])
                else:
                    nc.vector.scalar_tensor_tensor(
                        ob_out, ob_in, f_h.pop(b)[:], po[:], op0=ALU.mult, op1=ALU.add
                    )

            def ep(b):
                # epilogue: out = O * (NORM / rs) + NORM * bv
                rinv = stat.tile([P, 1], F32, tag="rinv", name="rinv")
                nc.vector.reciprocal(rinv[:], rs_out[:, b:b + 1])
                rn = stat.tile([P, 1], F32, tag="rn", name="rn")
                nc.vector.tensor_scalar(rn[:], rinv[:], NORM, None, op0=ALU.mult)
                of = outp.tile([P, VD], F32, tag="of", name=f"of{b}")
                nc.vector.scalar_tensor_tensor(
                    of[:], O_out[:, b, :], rn[:], bvN[:], op0=ALU.mult, op1=ALU.add
                )
                nc.scalar.dma_start(out_ap[b * P:(b + 1) * P, :], of[:])

            last = g == NG - 1

            def pv_ep(b):
                pv(b)
                if last:
                    ep(b)

            # slot b: [T(b-2), QK(b), PV(b-3)] — the transpose+eviction leads
            # the slot so the act-queue eviction clears before PV(b-2) needs
            # it next slot.  Slots 0-2 use leftover proj chunks as filler,
            # the group tail interleaves the next group's first proj chunks
            # so the PE has independent work while the last PVs drain.
            for b in range(NB):
                if b >= 2:
                    tr(b - 2)
                qk(b)
                if b < 3:
                    slot_fill[b]()
                if b >= 3:
                    pv_ep(b - 3)
            if g + 1 < NG:
                nxt_proj = make_proj(g + 1, *nxt)
                nch = nxt_proj[2]
                tr(NB - 2)
                pv_ep(NB - 3)
                nch[0]()
                tr(NB - 1)
                pv_ep(NB - 2)
                nch[1]()
                pv_ep(NB - 1)
                nch[2]()
                pending = nch[3:]
            else:
                tr(NB - 2)
                pv_ep(NB - 3)
                tr(NB - 1)
                pv_ep(NB - 2)
                pv_ep(NB - 1)


_cached = {}


def _build():
    if "nc" in _cached:
        return _cached["nc"]
    nc = bass.Bass("TRN2", target_bir_lowering=False, debug=False)
    aps = {
        "qT": nc.dram_tensor("qT", [P, 2, ND, 512], F16, kind="ExternalInput").ap(),
        "kT": nc.dram_tensor("kT", [P, NG, ND, GK], F16, kind="ExternalInput").ap(),
        "vT": nc.dram_tensor("vT", [P, NG, ND, GK], F16, kind="ExternalInput").ap(),
        "wqT": nc.dram_tensor("wqT", [P, ND, C], F16, kind="ExternalInput").ap(),
        "wkT": nc.dram_tensor("wkT", [P, ND, C], F16, kind="ExternalInput").ap(),
        "wvT": nc.dram_tensor("wvT", [P, ND, C], F16, kind="ExternalInput").ap(),
        "bqT": nc.dram_tensor("bqT", [P, NCC], F32, kind="ExternalInput").ap(),
        "bv": nc.dram_tensor("bv", [VD], F32, kind="ExternalInput").ap(),
        "ident": nc.dram_tensor("ident", [P, P], F16, kind="ExternalInput").ap(),
        "out": nc.dram_tensor("out", [RL, VD], F32, kind="ExternalOutput").ap(),
    }
    with tile.TileContext(nc) as tc:
        _emit(nc, tc, aps)
    _split_multi_waits(nc)
    _cached["nc"] = nc
    return nc


def kernel(q, k, v, Wq, bq, Wk, bk, Wv, bv, _trace=False, _tmpdir=None):
    del bk  # provably cancels inside the softmax
    nc = _build()

    def f16(a):
        return np.asarray(a, dtype=np.float32).astype(np.float16)

    def kv_prep(a):
        # [keys, d] -> [p, group, d-chunk, key-in-group]; 16KB/partition rows
        return np.ascontiguousarray(
            f16(a).reshape(NG, GK, ND, P).transpose(3, 0, 2, 1)
        )

    def w_prep(a):
        # [c, d] -> [p, d-chunk, c]
        return np.ascontiguousarray(f16(a).reshape(C, ND, P).transpose(2, 1, 0))

    q16 = f16(q)
    base = {
        "kT": kv_prep(k), "vT": kv_prep(v),
        "wqT": w_prep(Wq), "wkT": w_prep(Wk), "wvT": w_prep(Wv),
        "bqT": np.ascontiguousarray(
            np.asarray(bq, dtype=np.float32).reshape(NCC, P).T
        ),
        "bv": np.ascontiguousarray(np.asarray(bv, dtype=np.float32)),
        "ident": np.eye(P, dtype=np.float16),
    }
    in_maps = [
        dict(
            base,
            qT=np.ascontiguousarray(
                q16[c * RL:(c + 1) * RL].reshape(2, 512, ND, P).transpose(3, 0, 2, 1)
            ),
        )
        for c in range(NCORES)
    ]
    res = bass_utils.run_bass_kernel_spmd(
        nc, in_maps, core_ids=list(range(NCORES)), trace=_trace, tmpdir=_tmpdir
    )
    out = np.concatenate([res.results[c]["out"] for c in range(NCORES)], axis=0)
    if _trace:
        kernel.last_results = res
    return out


# revision 29
# speedup vs baseline: 2.8107x; 1.0376x over previous
"""Cross-attention kernel for Trainium2, sharded over 8 NeuronCores.

Shards query rows across cores (1024 rows each); K/V work is replicated.

Structure (flash-attention streaming, one pass over 8 key-groups of 1024):
  - Host pre-marshals inputs: fp32->fp16 cast + transpose into the exact
    [contraction-on-partition] layouts the PE needs.  The device does zero
    layout work for inputs; all FLOPs (projections + attention) stay on
    device.  Device DMA-in is ~41MB/core instead of the ~150MB a DRAM
    staging round-trip costs.
  - Per group g: load kT/vT slab (gpsimd SWDGE, double-buffered), project
    K^T [c,keys] and V [keys,v] (PE, evicted fp16 by the scalar engine),
    then stream attention for all 8 query row-blocks: S = Q@K^T (PSUM),
    online-softmax rescale (DVE/Act), A^T via PE transpose-mode matmuls
    into one fp16 PSUM bank (bulk-evicted by one DVE copy), and PV
    accumulated into per-block fp32 accumulators (flash rescale by
    exp(m_old - m_new) via one fused scalar_tensor_tensor).
  - The per-block stages are software-pipelined [QK(b), PV(b-2), T(b-1)]
    so the PE never waits for the softmax of the block it just produced.
  - PSUM budget exactly 8 banks: 3 proj/PV + 2x2 scores + 1 A^T.

Algebraic simplifications:
  - bk is dropped: it adds a per-row constant to scores, softmax cancels it.
  - bv is folded into the epilogue: atten rows sum to NORM after scaling,
    so out += NORM * bv.
  - softmax normalization and the post-softmax 1/sqrt(dk) scale fold into
    one per-row multiply at the end.
"""
import sys

sys.path.insert(0, "/opt/trn_rl_repo")

import numpy as np  # noqa: E402
import concourse.bass as bass  # noqa: E402
import concourse.tile as tile  # noqa: E402
from concourse import mybir  # noqa: E402
from concourse import bass_utils  # noqa: E402
from contextlib import ExitStack  # noqa: E402

F16 = mybir.dt.float16
F32 = mybir.dt.float32
AF = mybir.ActivationFunctionType
AX = mybir.AxisListType
ALU = mybir.AluOpType

P = 128
D = 1024             # input dim
ND = D // P          # 8 d-chunks
C = 512              # dim_k
NCC = C // P         # 4 c-chunks
VD = 512             # dim_v
KEYS = 8192
GK = 1024            # keys per group
NG = KEYS // GK      # 8 groups
RL = 1024            # query rows per core
NB = RL // P         # 8 row blocks
NCORES = 8
DEPTH = 3            # QK->PV software pipeline depth
NORM = float(1.0 / np.sqrt(np.float32(C)))

_ws_counter = [0]


def _split_multi_waits(nc):
    """This container's walrus accepts only ONE sync-wait per instruction.
    Move extra waits onto preceding same-engine EventSemaphore insts."""
    for f in nc.m.functions:
        for bb in f.blocks:
            il = bb.instructions
            if not any(
                inst.sync_info is not None and len(inst.sync_info.on_wait or ()) > 1
                for inst in il
            ):
                continue
            new = []
            for inst in il:
                si = inst.sync_info
                if si is not None and len(si.on_wait or ()) > 1:
                    waits = list(si.on_wait)
                    for w in waits[:-1]:
                        _ws_counter[0] += 1
                        new.append(
                            mybir.InstEventSemaphore(
                                name=f"I-ws{_ws_counter[0]}",
                                engine=inst.engine,
                                ins=[],
                                outs=[],
                                sync_info=mybir.SyncInfo(on_wait=[w], on_update=[]),
                            )
                        )
                    del si.on_wait[:-1]
                new.append(inst)
            bb.instructions = new


def _emit(nc, tc, aps):
    qT_r = aps["qT"]
    kT_r = aps["kT"]
    vT_r = aps["vT"]
    out_ap = aps["out"]

    with ExitStack() as top:
        const = top.enter_context(tc.tile_pool(name="const", bufs=1))
        kvin = top.enter_context(tc.tile_pool(name="kvin", bufs=2))
        proj = top.enter_context(tc.tile_pool(name="proj", bufs=2))
        apool = top.enter_context(tc.tile_pool(name="apool", bufs=4))
        atp = top.enter_context(tc.tile_pool(name="atp", bufs=6))
        stat = top.enter_context(tc.tile_pool(name="stat", bufs=6))
        outp = top.enter_context(tc.tile_pool(name="outp", bufs=2))
        pp = top.enter_context(tc.tile_pool(name="pp", bufs=3, space="PSUM"))
        psS = top.enter_context(tc.tile_pool(name="psS", bufs=2, space="PSUM"))
        pat = top.enter_context(tc.tile_pool(name="pat", bufs=1, space="PSUM"))

        # ---- persistent operands ----
        # sync queue: wq + qT halves first (they gate Q-proj), then wk/wv.
        wq = const.tile([P, ND, C], F16, tag="wq")
        wk = const.tile([P, ND, C], F16, tag="wk")
        wv = const.tile([P, ND, C], F16, tag="wv")
        nc.sync.dma_start(wq[:], aps["wqT"][:])
        qin = kvin.tile([P, 2, ND, 512], F16, tag="qin", bufs=1)
        nc.sync.dma_start(qin[:, 0], qT_r[:, 0])
        nc.sync.dma_start(qin[:, 1], qT_r[:, 1])
        kt0 = kvin.tile([P, ND, GK], F16, tag="kT", name="kTg0")
        vt0 = kvin.tile([P, ND, GK], F16, tag="vT", name="vTg0")
        nc.sync.dma_start(kt0[:], kT_r[:, 0, :, :])
        nc.sync.dma_start(wk[:], aps["wkT"][:])
        nc.sync.dma_start(vt0[:], vT_r[:, 0, :, :])
        nc.sync.dma_start(wv[:], aps["wvT"][:])
        bvrow = const.tile([1, VD], F32, tag="bvrow")
        nc.sync.dma_start(bvrow[:], aps["bv"][None, :])
        bqT = const.tile([P, NCC], F32, tag="bqT")
        nc.scalar.dma_start(bqT[:], aps["bqT"][:])
        ident = const.tile([P, P], F16, tag="ident")
        nc.scalar.dma_start(ident[:], aps["ident"][:])
        ones1 = const.tile([1, P], F32, tag="ones1")
        nc.vector.memset(ones1[:], 1.0)

        # Q^T projection: QT[c-chunk][128, rows] fp16, bias bq folded in.
        QT = const.tile([P, NCC, RL], F16, tag="QT")
        for rh in range(2):
            for ci in range(NCC):
                ps = pp.tile([P, 512], F32, tag="pp")
                for d in range(ND):
                    nc.tensor.matmul(
                        ps[:],
                        wq[:, d, ci * P:(ci + 1) * P],
                        qin[:, rh, d, :],
                        start=(d == 0),
                        stop=(d == ND - 1),
                    )
                nc.scalar.activation(
                    QT[:, ci, rh * 512:(rh + 1) * 512],
                    ps[:],
                    AF.Identity,
                    bias=bqT[:, ci:ci + 1],
                    scale=1.0,
                )

        # bvN[p, v] = NORM * bv[v] broadcast along partitions (rank-1 matmul)
        bvN = const.tile([P, VD], F32, tag="bvN")
        psb0 = pp.tile([P, VD], F32, tag="pp")
        nc.tensor.matmul(psb0[:], ones1[:], bvrow[:], start=True, stop=True)
        nc.scalar.activation(bvN[:], psb0[:], AF.Copy, bias=0.0, scale=NORM)

        # flash state, ping-pong by group parity: IN = st[g%2], OUT = st[1-g%2]
        m_st = [const.tile([P, NB], F32, tag=f"m{i}", name=f"m{i}") for i in range(2)]
        rs_st = [const.tile([P, NB], F32, tag=f"rs{i}", name=f"rs{i}") for i in range(2)]
        O_st = [
            const.tile([P, NB, VD], F32, tag=f"O{i}", name=f"O{i}") for i in range(2)
        ]

        def load_group(g):
            kt = kvin.tile([P, ND, GK], F16, tag="kT", name=f"kTg{g}")
            vt = kvin.tile([P, ND, GK], F16, tag="vT", name=f"vTg{g}")
            nc.gpsimd.dma_start(kt[:], kT_r[:, g, :, :])
            nc.gpsimd.dma_start(vt[:], vT_r[:, g, :, :])
            return kt, vt

        def make_proj(g, kt, vt):
            """K^T / V projection emitters for group g: 16 psum-group closures."""
            KT = proj.tile([P, NCC, GK], F16, tag="KT", name=f"KTg{g}")
            Vg = proj.tile([P, ND, VD], F16, tag="V", name=f"Vg{g}")
            chunks = []

            def k_chunk(ci, h):
                ps = pp.tile([P, 512], F32, tag="pp", name="psk")
                for d in range(ND):
                    nc.tensor.matmul(
                        ps[:],
                        wk[:, d, ci * P:(ci + 1) * P],
                        kt[:, d, h * 512:(h + 1) * 512],
                        start=(d == 0),
                        stop=(d == ND - 1),
                    )
                nc.scalar.copy(KT[:, ci, h * 512:(h + 1) * 512], ps[:])

            def v_chunk(kc):
                ps = pp.tile([P, 512], F32, tag="pp", name="psv")
                for d in range(ND):
                    nc.tensor.matmul(
                        ps[:],
                        vt[:, d, kc * P:(kc + 1) * P],
                        wv[:, d, :],
                        start=(d == 0),
                        stop=(d == ND - 1),
                    )
                nc.scalar.copy(Vg[:, kc, :], ps[:])

            for ci in range(NCC):
                for h in range(2):
                    chunks.append(lambda ci=ci, h=h: k_chunk(ci, h))
            for kc in range(8):
                chunks.append(lambda kc=kc: v_chunk(kc))
            return KT, Vg, chunks

        nxt = (kt0, vt0)
        nxt_proj = make_proj(0, *nxt)
        pending = nxt_proj[2]
        for g in range(NG):
            KT, Vg, _ = nxt_proj
            m_in, m_out = m_st[g % 2], m_st[1 - g % 2]
            rs_in, rs_out = rs_st[g % 2], rs_st[1 - g % 2]
            O_in, O_out = O_st[g % 2], O_st[1 - g % 2]

            # K^T / V projection chunks not emitted in g-1's tail: bulk now,
            # save the last three as PE filler for slots 0-2 (whose softmax
            # latency has no PV/T work to hide behind yet).
            for c in pending[:-3]:
                c()
            slot_fill = pending[-3:]
            # next group's load goes after this group's proj emission so the
            # prologue loads never compete with it for SDMA bandwidth
            if g + 1 < NG:
                nxt = load_group(g + 1)

            # ---- attention over this group, pipelined across row blocks ----
            a_h = {}
            at_h = {}
            f_h = {}

            def qk(b):
                S0 = psS.tile([P, 512], F32, tag="S0", name=f"S0_{g}_{b}")
                S1 = psS.tile([P, 512], F32, tag="S1", name=f"S1_{g}_{b}")
                for S, h in ((S0, 0), (S1, 1)):
                    for ci in range(NCC):
                        nc.tensor.matmul(
                            S[:],
                            QT[:, ci, b * P:(b + 1) * P],
                            KT[:, ci, h * 512:(h + 1) * 512],
                            start=(ci == 0),
                            stop=(ci == NCC - 1),
                        )
                gm0 = stat.tile([P, 1], F32, tag="gm0", name="gm0")
                gm1 = stat.tile([P, 1], F32, tag="gm1", name="gm1")
                nc.vector.reduce_max(gm0[:], S0[:], axis=AX.X)
                nc.vector.reduce_max(gm1[:], S1[:], axis=AX.X)
                mb = m_out[:, b:b + 1]
                if g == 0:
                    nc.vector.tensor_tensor(mb, gm0[:], gm1[:], op=ALU.max)
                else:
                    g01 = stat.tile([P, 1], F32, tag="g01", name="g01")
                    nc.vector.tensor_tensor(g01[:], gm0[:], gm1[:], op=ALU.max)
                    nc.vector.tensor_tensor(mb, m_in[:, b:b + 1], g01[:], op=ALU.max)
                negm = stat.tile([P, 1], F32, tag="negm", name="negm")
                nc.vector.tensor_scalar(negm[:], mb, -1.0, None, op0=ALU.mult)
                if g > 0:
                    f = stat.tile([P, 1], F32, tag="f", name="f")
                    nc.scalar.activation(
                        f[:], m_in[:, b:b + 1], AF.Exp, bias=negm[:], scale=1.0
                    )
                    f_h[b] = f
                A = apool.tile([P, GK], F16, tag="A", name=f"A_{g}_{b}")
                ps0 = stat.tile([P, 1], F32, tag="ps0", name="ps0")
                ps1 = stat.tile([P, 1], F32, tag="ps1", name="ps1")
                nc.scalar.activation(
                    A[:, 0:512], S0[:], AF.Exp, bias=negm[:], scale=1.0,
                    accum_out=ps0[:],
                )
                nc.scalar.activation(
                    A[:, 512:1024], S1[:], AF.Exp, bias=negm[:], scale=1.0,
                    accum_out=ps1[:],
                )
                rb_in, rb_out = rs_in[:, b:b + 1], rs_out[:, b:b + 1]
                if g == 0:
                    nc.vector.tensor_tensor(rb_out, ps0[:], ps1[:], op=ALU.add)
                else:
                    pss = stat.tile([P, 1], F32, tag="pss", name="pss")
                    nc.vector.tensor_tensor(pss[:], ps0[:], ps1[:], op=ALU.add)
                    nc.vector.scalar_tensor_tensor(
                        rb_out, rb_in, f_h[b][:], pss[:], op0=ALU.mult, op1=ALU.add
                    )
                a_h[b] = A

            def tr(b):
                # A^T for all 8 key chunks, PE transpose-mode -> one PSUM bank
                A = a_h.pop(b)
                tp = pat.tile([P, ND, P], F16, tag="tp", name=f"tp_{g}_{b}")
                for kc in range(8):
                    nc.tensor.transpose(
                        tp[:, kc, :], A[:, kc * P:(kc + 1) * P], ident[:]
                    )
                AT = atp.tile([P, ND, P], F16, tag="AT", name=f"AT_{g}_{b}")
                nc.scalar.copy(AT[:], tp[:])
                at_h[b] = AT

            def pv(b):
                po = pp.tile([P, VD], F32, tag="pp", name=f"po_{g}_{b}")
                AT = at_h.pop(b)
                for kc in range(8):
                    nc.tensor.matmul(
                        po[:],
                        AT[:, kc, :],
                        Vg[:, kc, :],
                        start=(kc == 0),
                        stop=(kc == 7),
                    )
                ob_out, ob_in = O_out[:, b, :], O_in[:, b, :]
                if g == 0:
                    nc.vector.tensor_copy(ob_out, po[:])
                else:
                    nc.vector.scalar_tensor_tensor(
                        ob_out, ob_in, f_h.pop(b)[:], po[:], op0=ALU.mult, op1=ALU.add
                    )

            def ep(b):
                # epilogue: out = O * (NORM / rs) + NORM * bv
                rinv = stat.tile([P, 1], F32, tag="rinv", name="rinv")
                nc.vector.reciprocal(rinv[:], rs_out[:, b:b + 1])
                rn = stat.tile([P, 1], F32, tag="rn", name="rn")
                nc.vector.tensor_scalar(rn[:], rinv[:], NORM, None, op0=ALU.mult)
                of = outp.tile([P, VD], F32, tag="of", name=f"of{b}")
                nc.vector.scalar_tensor_tensor(
                    of[:], O_out[:, b, :], rn[:], bvN[:], op0=ALU.mult, op1=ALU.add
                )
                nc.scalar.dma_start(out_ap[b * P:(b + 1) * P, :], of[:])

            last = g == NG - 1

            def pv_ep(b):
                pv(b)
                if last:
                    ep(b)

            # slot b: [T(b-2), QK(b), PV(b-3)] — the transpose+eviction leads
            # the slot so the act-queue eviction clears before PV(b-2) needs
            # it next slot.  Slots 0-2 use leftover proj chunks as filler,
            # the group tail interleaves the next group's first proj chunks
            # so the PE has independent work while the last PVs drain.
            for b in range(NB):
                if b >= 2:
                    tr(b - 2)
                qk(b)
                if b < 3:
                    slot_fill[b]()
                if b >= 3:
                    pv_ep(b - 3)
            if g + 1 < NG:
                nxt_proj = make_proj(g + 1, *nxt)
                nch = nxt_proj[2]
                tr(NB - 2)
                pv_ep(NB - 3)
                nch[0]()
                tr(NB - 1)
                pv_ep(NB - 2)
                nch[1]()
                pv_ep(NB - 1)
                nch[2]()
                pending = nch[3:]
            else:
                tr(NB - 2)
                pv_ep(NB - 3)
                tr(NB - 1)
                pv_ep(NB - 2)
                pv_ep(NB - 1)


_cached = {}


def _build():
    if "nc" in _cached:
        return _cached["nc"]
    nc = bass.Bass("TRN2", target_bir_lowering=False, debug=False)
    aps = {
        "qT": nc.dram_tensor("qT", [P, 2, ND, 512], F16, kind="ExternalInput").ap(),
        "kT": nc.dram_tensor("kT", [P, NG, ND, GK], F16, kind="ExternalInput").ap(),
        "vT": nc.dram_tensor("vT", [P, NG, ND, GK], F16, kind="ExternalInput").ap(),
        "wqT": nc.dram_tensor("wqT", [P, ND, C], F16, kind="ExternalInput").ap(),
        "wkT": nc.dram_tensor("wkT", [P, ND, C], F16, kind="ExternalInput").ap(),
        "wvT": nc.dram_tensor("wvT", [P, ND, C], F16, kind="ExternalInput").ap(),
        "bqT": nc.dram_tensor("bqT", [P, NCC], F32, kind="ExternalInput").ap(),
        "bv": nc.dram_tensor("bv", [VD], F32, kind="ExternalInput").ap(),
        "ident": nc.dram_tensor("ident", [P, P], F16, kind="ExternalInput").ap(),
        "out": nc.dram_tensor("out", [RL, VD], F32, kind="ExternalOutput").ap(),
    }
    with tile.TileContext(nc) as tc:
        _emit(nc, tc, aps)
    _split_multi_waits(nc)
    _cached["nc"] = nc
    return nc


def kernel(q, k, v, Wq, bq, Wk, bk, Wv, bv, _trace=False, _tmpdir=None):
    del bk  # provably cancels inside the softmax
    nc = _build()

    def f16(a):
        return np.asarray(a, dtype=np.float32).astype(np.float16)

    def kv_prep(a):
        # [keys, d] -> [p, group, d-chunk, key-in-group]; 16KB/partition rows
        return np.ascontiguousarray(
            f16(a).reshape(NG, GK, ND, P).transpose(3, 0, 2, 1)
        )

    def w_prep(a):
        # [c, d] -> [p, d-chunk, c]
        return np.ascontiguousarray(f16(a).reshape(C, ND, P).transpose(2, 1, 0))

    q16 = f16(q)
    base = {
        "kT": kv_prep(k), "vT": kv_prep(v),
        "wqT": w_prep(Wq), "wkT": w_prep(Wk), "wvT": w_prep(Wv),
        "bqT": np.ascontiguousarray(
            np.asarray(bq, dtype=np.float32).reshape(NCC, P).T
        ),
        "bv": np.ascontiguousarray(np.asarray(bv, dtype=np.float32)),
        "ident": np.eye(P, dtype=np.float16),
    }
    in_maps = [
        dict(
            base,
            qT=np.ascontiguousarray(
                q16[c * RL:(c + 1) * RL].reshape(2, 512, ND, P).transpose(3, 0, 2, 1)
            ),
        )
        for c in range(NCORES)
    ]
    res = bass_utils.run_bass_kernel_spmd(
        nc, in_maps, core_ids=list(range(NCORES)), trace=_trace, tmpdir=_tmpdir
    )
    out = np.concatenate([res.results[c]["out"] for c in range(NCORES)], axis=0)
    if _trace:
        kernel.last_results = res
    return out
